# revision 3
# baseline (speedup 1.0000x reference)
"""Trainium2 Bass kernel for nn_DeliveryEventEncoder — v2 "packed tiles".

Data-parallel over 8 cores (128 units/core). Per core, units are sorted by
length and tile-quantized: unit u owns kt=ceil(len/128) in {1,2} token tiles;
only those TT = sum(kt) ~ 1.5*NU tiles (vs 2*NU dense) are processed. The
module is specialized on the observed lengths (rebuilt per kernel() call);
all 8 cores share one SPMD module sized to the max long-count across cores.

Key structural choices vs the per-unit baseline:
- Units are grouped into chunks of <=4 tiles; all elementwise work runs on
  [128, 512]-wide tiles (one PSUM bank), amortizing fixed engine latencies.
- Two mega-phases split by ACT table: phase 1 (attention, exp) for all
  chunks, then phase 2 (LayerNorm sqrt, FFN) -> exactly 2 table loads.
- softmax reciprocal eliminated: LN(emb + ao/den) == LN(den*emb + ao) by
  LayerNorm scale invariance, so the denominator multiplies the embedding
  (per-partition scalar in token-major layout) instead of dividing ao.
- key masking is folded into the exp: a rank-1 PE matmul adds -400 to
  invalid-key score rows, so exp gives exact zeros; query masking folds
  into LN2's rstd (batched, one op per supergroup).
- LN stats come for free from accum_out on the residual-add/square ops and
  are post-processed in a few [128, TT]-wide batched ops.
"""

import os
import numpy as np
import ml_dtypes

import concourse.bass as bass
import concourse.bacc as bacc_mod
import concourse.mybir as mybir
import concourse.tile as tile
from concourse.bass_utils import run_bass_kernel_spmd
from concourse.masks import make_identity

F32 = mybir.dt.float32
BF16 = mybir.dt.bfloat16
AF = mybir.ActivationFunctionType
ALU = mybir.AluOpType
NPBF = ml_dtypes.bfloat16

B, U, L, DSEQ, H, DOUT = 32, 32, 256, 5, 128, 128
TODV, TODD, AGGD, UNITD = 5, 3, 7, 16
NCORES = 8
BPC = B // NCORES          # buildings per core
NU = BPC * U               # units per core (128)
CSCALE = 1.0 / np.sqrt(H)
EPS = 1e-5
NEGB = -400.0              # pre-scale score bias for invalid keys


def _chunks_of(kt):
    """Greedy chunks of units with sum(kt) <= 4 tiles each."""
    chunks, cur, cnt = [], [], 0
    for u in range(len(kt)):
        if cnt + kt[u] > 4:
            chunks.append(cur)
            cur, cnt = [], 0
        cur.append(u)
        cnt += kt[u]
    if cur:
        chunks.append(cur)
    return chunks


def build_nc(wts):
    wts = dict(wts)
    nlong = int(wts.pop("_nlong"))
    kt = [1] * (NU - nlong) + [2] * nlong
    tstart = np.concatenate([[0], np.cumsum(kt)]).astype(int)  # tile index per unit
    TT = int(tstart[NU])
    chunks = _chunks_of(kt)
    NCH = len(chunks)

    SGC = int(os.environ.get("KSGC", "16"))
    routes = dict(embT="dve", yT="act", vs="act", en="mm", enc="dve",
                  stt1="dve", sq1="dve", norm1="dve", x1t="dma",
                  x1tc="dve", relu="act", x2c="act", sq2="dve")
    for kv in os.environ.get("KROUTE", "").split(","):
        if kv:
            k_, v_ = kv.split("=")
            routes[k_] = v_

    nc = bacc_mod.Bacc()

    x_in = nc.dram_tensor("xg", [DSEQ, TT * 128], BF16, kind="ExternalInput")
    NB3 = (TT + 2) // 3
    negb_in = nc.dram_tensor("negbT", [65, NB3 * 128], BF16, kind="ExternalInput")
    qm_in = nc.dram_tensor("qm01", [128, 256], F32, kind="ExternalInput")
    s_in = nc.dram_tensor("S", [NU, BPC], BF16, kind="ExternalInput")
    tail_in = nc.dram_tensor("tail", [AGGD + TODD, BPC], BF16, kind="ExternalInput")
    out_t = nc.dram_tensor("outT", [DOUT, BPC], F32, kind="ExternalOutput")

    dW = {k: nc.inline_tensor(v, name=k) for k, v in wts.items()}

    with tile.TileContext(nc) as tc:
        with (
            tc.tile_pool(name="singles", bufs=1) as singles,
            tc.tile_pool(name="xpool", bufs=4) as xpool,
            tc.tile_pool(name="cp", bufs=int(os.environ.get("KCP", "8"))) as cp,
            tc.tile_pool(name="esp", bufs=int(os.environ.get("KESP", "8"))) as esp,
            tc.tile_pool(name="wk2", bufs=int(os.environ.get("KWK", "8"))) as wk2,
            tc.tile_pool(name="pA", bufs=int(os.environ.get("KPA", "4")), space="PSUM") as pA,
            tc.tile_pool(name="pB", bufs=int(os.environ.get("KPB", "2")), space="PSUM") as pB,
            tc.tile_pool(name="pTT", bufs=1, space="PSUM") as pTT,
            tc.tile_pool(name="pS", bufs=2, space="PSUM") as pS,    # small (den/pool)
        ):
            # ---- constants ----
            def load_w(name, p, f):
                t = singles.tile([p, f], BF16, tag=name)
                nc.sync.dma_start(out=t, in_=dW[name][:, :])
                return t

            w_in = load_w("w_inT", DSEQ, H)
            w_g = load_w("w_gT", H, H)
            w_vo = load_w("w_voT", H, H)
            w_f1 = load_w("w_f1T", H, H)
            w_f2 = load_w("w_f2T", H, H)
            c2col = load_w("w_c2col", H, 1)
            w_u = load_w("w_uT", H, UNITD)
            w_c1 = load_w("w_c1T", UNITD + AGGD + TODD, H)
            w_c2 = load_w("w_c2T", H, DOUT)

            identb = singles.tile([128, 128], BF16, tag="identb")
            make_identity(nc, identb)
            identf = singles.tile([128, 128], F32, tag="identf")
            make_identity(nc, identf)
            ones_col = singles.tile([128, 1], BF16, tag="ones_col")
            nc.vector.memset(ones_col, 1.0)
            ones_mat = singles.tile([128, 128], BF16, tag="ones_mat")
            nc.vector.memset(ones_mat, 1.0)
            ones_row = singles.tile([65, 512], BF16, tag="ones_row")
            nc.vector.memset(ones_row, 1.0)
            eps_col = singles.tile([128, 1], F32, tag="eps")
            nc.vector.memset(eps_col, EPS)

            negb_sb = singles.tile([65, NB3 * 128], BF16, tag="negb")
            nc.sync.dma_start(out=negb_sb, in_=negb_in[:, :])
            qm_sb = singles.tile([128, 256], F32, tag="qm")
            nc.sync.dma_start(out=qm_sb, in_=qm_in[:, :])
            s_sb = singles.tile([NU, BPC], BF16, tag="S")
            nc.sync.dma_start(out=s_sb, in_=s_in[:, :])

            # big persistent activations / stats
            x1in_all = singles.tile([128, TT * 128], BF16, tag="x1in")
            x2in_sg = singles.tile([128, SGC * 512], BF16, tag="x2in")
            s1g = singles.tile([128, 256], F32, tag="s1g")
            q1g = singles.tile([128, 256], F32, tag="q1g")
            s2g = singles.tile([128, 256], F32, tag="s2g")
            q2g = singles.tile([128, 256], F32, tag="q2g")
            mean1 = singles.tile([128, 256], F32, tag="mean1")
            rstd1 = singles.tile([128, 256], F32, tag="rstd1")
            mb1 = singles.tile([128, 256], F32, tag="mb1")
            mean2 = singles.tile([128, 256], F32, tag="mean2")
            r2mb = singles.tile([128, 256], BF16, tag="r2mb")
            nmrm = singles.tile([128, 256], BF16, tag="nmrm")
            rstd2m = singles.tile([128, 256], F32, tag="rstd2m")
            tmpa = singles.tile([128, 256], F32, tag="tmpa")
            tmpb = singles.tile([128, 256], F32, tag="tmpb")
            junk = singles.tile([128, 512], BF16, tag="junk")
            pooled = singles.tile([H, NU], BF16, tag="pooled")

            mm = nc.tensor.matmul

            # ---- engine routing (tunable via KROUTE env) ----
            ENG = {"dve": nc.vector, "pool": nc.gpsimd}

            def rt_copy(kind, out, in_):
                r = routes[kind]
                if r == "act":
                    nc.scalar.activation(out=out, in_=in_, func=AF.Copy,
                                         bias=0.0, scale=1.0)
                else:
                    ENG[r].tensor_copy(out, in_)

            def rt_stt(kind, **kw):
                r = routes[kind]
                if r == "act" and kw.get("op1") == ALU.mult:
                    nc.scalar.activation(out=kw["out"], in_=kw["in0"],
                                         func=AF.Square, bias=0.0, scale=1.0,
                                         accum_out=kw["accum_out"])
                else:
                    ENG[r].scalar_tensor_tensor(**kw)

            def rt_norm(kind, out, in_, mean_c, rstd_c, mbias_c):
                r = routes[kind]
                if r == "act":
                    nc.scalar.activation(out=out, in_=in_, func=AF.Identity,
                                         bias=mbias_c, scale=rstd_c)
                else:
                    ENG[r].tensor_scalar(out=out, in0=in_, scalar1=mean_c,
                                         scalar2=rstd_c, op0=ALU.subtract,
                                         op1=ALU.mult)

            def rt_relu(kind, out, in_):
                r = routes[kind]
                if r == "act":
                    nc.scalar.activation(out=out, in_=in_, func=AF.Relu,
                                         bias=0.0, scale=1.0)
                else:
                    ENG[r].tensor_scalar_max(out=out, in0=in_, scalar1=0.0)

            # chunk geometry helpers
            def cgeom(ch):
                """units with local tile offsets; returns (units, w) where
                units = [(u, base_tile_global, base_local)] and w = #tiles."""
                units = []
                loc = 0
                for u in ch:
                    units.append((u, int(tstart[u]), loc))
                    loc += kt[u]
                return units, loc

            def boundary(u, j):
                """tile j of unit u is its last tile (may be ragged)."""
                return j == kt[u] - 1

            def negb_row(t):
                p = 32 * (t % 3)
                c = (t // 3) * 128
                return negb_sb[p:p + 1, c:c + 128]

            # ================= phase 1 chunk body (exp table) =================
            def emit_p1(ch):
                units, w = cgeom(ch)
                W = w * 128
                t0 = int(tstart[ch[0]])

                xs = xpool.tile([DSEQ, 512], BF16, tag="xs")
                nc.sync.dma_start(out=xs[:, :W], in_=x_in[:, t0 * 128:(t0 + w) * 128])

                emb_ps = pA.tile([128, 512], F32, tag="pA")
                mm(emb_ps[:, :W], w_in, xs[:, :W], start=True, stop=True)
                embT = cp.tile([128, 512], BF16, tag="embT")
                rt_copy("embT", embT[:, :W], emb_ps[:, :W])

                if routes["en"] == "dma":
                    en_sb = cp.tile([128, 512], BF16, tag="en")
                    nc.sync.dma_start_transpose(
                        en_sb[:, :W].rearrange("t (b h) -> t b h", b=w),
                        embT[:, :W])
                else:
                    en_ps = pB.tile([128, 512], F32, tag="pB")
                    for j in range(w):
                        mm(en_ps[:, j * 128:(j + 1) * 128],
                           xs[:, j * 128:(j + 1) * 128], w_in,
                           start=True, stop=True)
                    en_sb = cp.tile([128, 512], BF16, tag="en")
                    rt_copy("enc", en_sb[:, :W], en_ps[:, :W])

                y_ps = pA.tile([128, 512], F32, tag="pA")
                mm(y_ps[:, :W], w_g, embT[:, :W], start=True, stop=True)
                yT = cp.tile([128, 512], BF16, tag="yT")
                rt_copy("yT", yT[:, :W], y_ps[:, :W])

                v_ps = pA.tile([128, 512], F32, tag="pA")
                for j in range(w):
                    mm(v_ps[:, j * 128:(j + 1) * 128],
                       embT[:, j * 128:(j + 1) * 128], w_vo, start=True, stop=True)
                vs = cp.tile([128, 512], BF16, tag="vs")
                rt_copy("vs", vs[:, :W], v_ps[:, :W])

                # scores + exp, one bank per key-tile index (mt)
                maxkt = max(kt[u] for u in ch)
                es_mt = []
                for mt in range(maxkt):
                    # column layout within this mt-bank: q-ranges of units with kt>mt
                    sc_ps = pA.tile([128, 512], F32, tag="pA")
                    col = 0
                    spans = []  # (u, base_local, qw, col)
                    for (u, bg, bl) in units:
                        if kt[u] <= mt:
                            continue
                        qw = kt[u] * 128
                        lhs = embT[:, (bl + mt) * 128:(bl + mt + 1) * 128]
                        bnd = boundary(u, mt)
                        mm(sc_ps[:, col:col + qw], lhs,
                           yT[:, bl * 128:bl * 128 + qw],
                           start=True,
                           stop=(bnd and bool(os.environ.get("KNONEGB"))) or not bnd)
                        if bnd and not os.environ.get("KNONEGB"):
                            p3 = 32 * ((bg + mt) % 3)
                            mm(sc_ps[:, col:col + qw], negb_row(bg + mt),
                               ones_row[p3:p3 + 1, :qw], start=False, stop=True)
                        spans.append((u, bl, qw, col))
                        col += qw
                    es = esp.tile([128, 512], BF16, tag=f"es{mt}")
                    nc.scalar.activation(out=es[:, :col], in_=sc_ps[:, :col],
                                         func=AF.Exp, bias=0.0, scale=CSCALE)
                    es_mt.append((es, spans))

                # fused: den (exp row-sums) and ao_proj = es.T @ (emb @ Wv Wo)
                # share the same stationary es tile per (unit, qtile, ktile)
                den_ps = pS.tile([128, 16], F32, tag="sm")
                pon_ps = pB.tile([128, 512], F32, tag="pB")
                for (u, bg, bl) in units:
                    for j in range(kt[u]):
                        for mt in range(kt[u]):
                            es, spans = es_mt[mt]
                            ucol = next(c for (uu, _, _, c) in spans if uu == u)
                            lhs = es[:, ucol + j * 128:ucol + (j + 1) * 128]
                            st, sp = (mt == 0), (mt == kt[u] - 1)
                            mm(den_ps[:, bl + j:bl + j + 1], lhs, ones_col,
                               start=st, stop=sp)
                            mm(pon_ps[:, (bl + j) * 128:(bl + j + 1) * 128],
                               lhs, vs[:, (bl + mt) * 128:(bl + mt + 1) * 128],
                               start=st, stop=sp)

                den_sb = cp.tile([128, 16], F32, tag="den_sb")
                nc.vector.tensor_copy(den_sb[:, :w], den_ps[:, :w])
                # x1in = den*emb + ao_proj  (token-major), stats via accum
                for j in range(w):
                    t = t0 + j
                    xsl = x1in_all[:, t * 128:(t + 1) * 128]
                    rt_stt("stt1", out=xsl, in0=en_sb[:, j * 128:(j + 1) * 128],
                           scalar=den_sb[:, j:j + 1],
                           in1=pon_ps[:, j * 128:(j + 1) * 128],
                           op0=ALU.mult, op1=ALU.add,
                           accum_out=s1g[:, t:t + 1])
                    rt_stt("sq1", out=junk[:, :128], in0=xsl, scalar=1.0, in1=xsl,
                           op0=ALU.mult, op1=ALU.mult,
                           accum_out=q1g[:, t:t + 1])

            # ---- batched LN1 stats: rstd1, mb1 = -mean*rstd ----
            def emit_stats1(ta, tb):
                nc.vector.tensor_scalar(out=mean1[:, ta:tb], in0=s1g[:, ta:tb],
                                        scalar1=1.0 / H, scalar2=None, op0=ALU.mult)
                nc.vector.tensor_tensor(out=tmpa[:, ta:tb], in0=mean1[:, ta:tb],
                                        in1=mean1[:, ta:tb], op=ALU.mult)
                nc.vector.scalar_tensor_tensor(
                    out=tmpb[:, ta:tb], in0=q1g[:, ta:tb], scalar=1.0 / H,
                    in1=tmpa[:, ta:tb], op0=ALU.mult, op1=ALU.subtract)
                nc.scalar.activation(out=tmpa[:, ta:tb], in_=tmpb[:, ta:tb],
                                     func=AF.Sqrt, bias=eps_col, scale=1.0)
                nc.vector.reciprocal(rstd1[:, ta:tb], tmpa[:, ta:tb])
                nc.vector.scalar_tensor_tensor(
                    out=mb1[:, ta:tb], in0=mean1[:, ta:tb], scalar=-1.0,
                    in1=rstd1[:, ta:tb], op0=ALU.mult, op1=ALU.mult)

            # ================= phase 2 supergroup (sqrt table) =================
            def emit_p2(sg0):
                sgch = chunks[sg0:sg0 + SGC]
                # B2: normalize, FFN, residual + stats
                for si, ch in enumerate(sgch):
                    units, w = cgeom(ch)
                    W = w * 128
                    t0 = int(tstart[ch[0]])
                    x1 = wk2.tile([128, 512], BF16, tag="x1")
                    for j in range(w):
                        t = t0 + j
                        rt_norm("norm1", x1[:, j * 128:(j + 1) * 128],
                                x1in_all[:, t * 128:(t + 1) * 128],
                                mean1[:, t:t + 1], rstd1[:, t:t + 1],
                                mb1[:, t:t + 1])
                    x1T = wk2.tile([128, 512], BF16, tag="x1T")
                    if routes.get("x1t", "dma") == "dma":
                        nc.sync.dma_start_transpose(
                            x1T[:, :W].rearrange("h (b t) -> h b t", b=w),
                            x1[:, :W])
                    else:
                        x1t_ps = pTT.tile([128, 512], BF16, tag="pBt")
                        for j in range(w):
                            nc.tensor.transpose(
                                x1t_ps[:, j * 128:(j + 1) * 128],
                                x1[:, j * 128:(j + 1) * 128], identb)
                        rt_copy("x1tc", x1T[:, :W], x1t_ps[:, :W])

                    f1_ps = pA.tile([128, 512], F32, tag="pA")
                    mm(f1_ps[:, :W], w_f1, x1T[:, :W], start=True, stop=True)
                    f1r = wk2.tile([128, 512], BF16, tag="f1r")
                    rt_relu("relu", f1r[:, :W], f1_ps[:, :W])

                    f2_ps = pB.tile([128, 512], F32, tag="pB")
                    s2_ps = pS.tile([128, 16], F32, tag="sm")
                    for j in range(w):
                        f1sl = f1r[:, j * 128:(j + 1) * 128]
                        x1sl = x1T[:, j * 128:(j + 1) * 128]
                        mm(f2_ps[:, j * 128:(j + 1) * 128], f1sl, w_f2,
                           start=True, stop=False)
                        mm(f2_ps[:, j * 128:(j + 1) * 128], x1sl, identb,
                           start=False, stop=True)
                        mm(s2_ps[:, j:j + 1], f1sl, c2col, start=True, stop=False)
                        mm(s2_ps[:, j:j + 1], x1sl, ones_col, start=False, stop=True)
                    nc.vector.tensor_copy(s2g[:, t0:t0 + w], s2_ps[:, :w])
                    xsl = x2in_sg[:, si * 512:si * 512 + W]
                    rt_copy("x2c", xsl, f2_ps[:, :W])
                    for j in range(w):
                        t = t0 + j
                        xj = x2in_sg[:, (si * 4 + j) * 128:(si * 4 + j + 1) * 128]
                        rt_stt("sq2", out=junk[:, :128], in0=xj, scalar=1.0,
                               in1=xj, op0=ALU.mult, op1=ALU.mult,
                               accum_out=q2g[:, t:t + 1])

                # batched LN2 stats for this supergroup
                ta = int(tstart[sgch[0][0]])
                tb = int(tstart[sgch[-1][-1]]) + kt[sgch[-1][-1]]
                nt = tb - ta
                nc.vector.tensor_scalar(out=mean2[:, ta:tb], in0=s2g[:, ta:tb],
                                        scalar1=1.0 / H, scalar2=None, op0=ALU.mult)
                nc.vector.tensor_tensor(out=tmpa[:, ta:tb], in0=mean2[:, ta:tb],
                                        in1=mean2[:, ta:tb], op=ALU.mult)
                nc.vector.scalar_tensor_tensor(
                    out=tmpb[:, ta:tb], in0=q2g[:, ta:tb], scalar=1.0 / H,
                    in1=tmpa[:, ta:tb], op0=ALU.mult, op1=ALU.subtract)
                nc.scalar.activation(out=tmpa[:, ta:tb], in_=tmpb[:, ta:tb],
                                     func=AF.Sqrt, bias=eps_col, scale=1.0)
                nc.vector.reciprocal(tmpb[:, ta:tb], tmpa[:, ta:tb])
                nc.vector.tensor_tensor(out=rstd2m[:, ta:tb], in0=tmpb[:, ta:tb],
                                        in1=qm_sb[:, ta:tb], op=ALU.mult)
                nc.vector.tensor_copy(r2mb[:, ta:tb], rstd2m[:, ta:tb])
                nc.vector.scalar_tensor_tensor(
                    out=nmrm[:, ta:tb], in0=mean2[:, ta:tb], scalar=-1.0,
                    in1=rstd2m[:, ta:tb], op0=ALU.mult, op1=ALU.mult)

                # B3: normalize + masked sum-pool
                for si, ch in enumerate(sgch):
                    units, w = cgeom(ch)
                    t0 = int(tstart[ch[0]])
                    pool_ps = pS.tile([128, 16], F32, tag="sm")
                    for ui, (u, bg, bl) in enumerate(units):
                        for j in range(kt[u]):
                            t = t0 + bl + j
                            x2sl = x2in_sg[:, (si * 4 + bl + j) * 128:
                                           (si * 4 + bl + j + 1) * 128]
                            mm(pool_ps[:, ui:ui + 1], x2sl, r2mb[:, t:t + 1],
                               start=(j == 0), stop=False)
                            mm(pool_ps[:, ui:ui + 1], ones_mat,
                               nmrm[:, t:t + 1],
                               start=False, stop=(j == kt[u] - 1))
                    nc.vector.tensor_copy(
                        pooled[:, ch[0]:ch[0] + len(units)],
                        pool_ps[:, :len(units)])

            # ================= interleaved driver =================
            def sg_trange(sg0):
                sgch = chunks[sg0:sg0 + SGC]
                ta = int(tstart[sgch[0][0]])
                tb = int(tstart[sgch[-1][-1]]) + kt[sgch[-1][-1]]
                return ta, tb

            LAG = int(os.environ.get("KLAG", "0"))
            sgs = list(range(0, NCH, SGC))
            if LAG == 0:
                for ch in chunks:
                    emit_p1(ch)
                for sg0 in sgs:
                    emit_stats1(*sg_trange(sg0))
                    emit_p2(sg0)
            else:
                for i, sg0 in enumerate(sgs):
                    for ch in chunks[sg0:sg0 + SGC]:
                        emit_p1(ch)
                    if i >= LAG:
                        prev = sgs[i - LAG]
                        emit_stats1(*sg_trange(prev))
                        emit_p2(prev)
                for i in range(max(0, len(sgs) - LAG), len(sgs)):
                    emit_stats1(*sg_trange(sgs[i]))
                    emit_p2(sgs[i])

            # ================= tail: unit_fc, building sum, fusion =================
            u16_ps = pB.tile([UNITD, NU], F32, tag="pB")
            mm(u16_ps, w_u, pooled, start=True, stop=True)
            u16 = cp.tile([UNITD, NU], F32, tag="u16")
            nc.scalar.activation(out=u16, in_=u16_ps, func=AF.Relu,
                                 bias=0.0, scale=1.0)

            u16t_ps = pB.tile([NU, UNITD], F32, tag="pB")
            nc.tensor.transpose(u16t_ps, u16, identf[:UNITD, :UNITD])
            u16t = cp.tile([NU, UNITD], BF16, tag="u16t")
            nc.vector.tensor_copy(u16t, u16t_ps)

            seq_ps = pB.tile([UNITD, BPC], F32, tag="pB")
            mm(seq_ps, u16t, s_sb, start=True, stop=True)

            fused = cp.tile([UNITD + AGGD + TODD, BPC], BF16, tag="fused")
            nc.vector.tensor_copy(fused[:UNITD, :], seq_ps)
            nc.sync.dma_start(out=fused[UNITD:, :], in_=tail_in[:, :])

            h1_ps = pB.tile([H, BPC], F32, tag="pB")
            mm(h1_ps, w_c1, fused, start=True, stop=True)
            h1 = cp.tile([H, BPC], BF16, tag="h1")
            nc.scalar.activation(out=h1, in_=h1_ps, func=AF.Relu,
                                 bias=0.0, scale=1.0)

            o_ps = pB.tile([DOUT, BPC], F32, tag="pB")
            mm(o_ps, w_c2, h1, start=True, stop=True)
            o_s = cp.tile([DOUT, BPC], F32, tag="osb")
            nc.scalar.activation(out=o_s, in_=o_ps, func=AF.Relu,
                                 bias=0.0, scale=1.0)
            nc.sync.dma_start(out=out_t[:, :], in_=o_s)

    return nc


def _prep_weights(inputs):
    ipw = np.asarray(inputs["in_proj_w"])
    wts = {
        "w_inT": np.asarray(inputs["W_in"]).T,        # [5,128]
        "w_gT": (ipw[0:H] @ ipw[H:2 * H].T),          # composed q/k [128,128]
        "w_voT": ipw[2 * H:3 * H].T @ np.asarray(inputs["out_proj_w"]).T,
        "w_f1T": np.asarray(inputs["W_ff1"]).T,
        "w_f2T": np.asarray(inputs["W_ff2"]).T,
        "w_c2col": np.asarray(inputs["W_ff2"]).T.sum(axis=1, keepdims=True),
        "w_uT": np.asarray(inputs["W_unit"]).T,       # [128,16]
        "w_c1T": np.asarray(inputs["W_fc1"]).T,       # [26,128]
        "w_c2T": np.asarray(inputs["W_fc2"]).T,       # [128,128]
    }
    wts = {k: np.ascontiguousarray(v.astype(NPBF)) for k, v in wts.items()}
    for nm in ("b_in", "in_proj_b", "out_proj_b", "b_ff1", "b_ff2",
               "ln1_b", "ln2_b", "b_unit", "b_fc1", "b_fc2"):
        assert np.max(np.abs(np.asarray(inputs[nm]))) == 0.0, f"{nm} nonzero"
    for nm in ("ln1_w", "ln2_w"):
        assert np.allclose(np.asarray(inputs[nm]), 1.0), f"{nm} nontrivial"

    lengths = np.asarray(inputs["lengths"]).reshape(NCORES, NU)
    nlong = int(max((lengths[c] > 128).sum() for c in range(NCORES)))
    wts["_nlong"] = nlong
    return wts


def make_in_maps(inputs, nlong):
    x_seq = np.asarray(inputs["x_seq"], dtype=np.float32)        # [B,U,L,5]
    lengths = np.asarray(inputs["lengths"]).reshape(NCORES, NU)
    x_agg = np.asarray(inputs["x_agg_quant"], dtype=np.float32)  # [B,7]
    tod_emb = np.asarray(inputs["tod_emb"], dtype=np.float32)    # [5,3]
    tod_idx = np.asarray(inputs["tod_idx"])                      # [B]

    kt_mod = np.array([1] * (NU - nlong) + [2] * nlong)
    tstart = np.concatenate([[0], np.cumsum(kt_mod)]).astype(int)
    TT = int(tstart[NU])

    in_maps = []
    for c in range(NCORES):
        lens = lengths[c]
        xc = x_seq[c * BPC:(c + 1) * BPC].reshape(NU, L, DSEQ)
        # sort units: shorts (len<=128) first
        order = np.argsort(lens > 128, kind="stable")
        xg = np.zeros((DSEQ, TT * 128), np.float32)
        NB3 = (TT + 2) // 3
        negbT = np.zeros((65, NB3 * 128), np.float32)
        qm01 = np.zeros((128, 256), np.float32)
        for i in range(NU):
            u = order[i]
            ln = int(lens[u])
            t0 = int(tstart[i])
            ntile = int(kt_mod[i])
            for j in range(ntile):
                t = t0 + j
                lo = j * 128
                valid = max(0, min(128, ln - lo))
                if valid > 0:
                    xg[:, t * 128:t * 128 + valid] = \
                        xc[u, lo:lo + valid, :].T
                negbT[32 * (t % 3), (t // 3) * 128 + valid:(t // 3 + 1) * 128] = NEGB
                qm01[:valid, t] = 1.0
        S = np.zeros((NU, BPC), np.float32)
        S[np.arange(NU), order // U] = 1.0
        tail = np.concatenate(
            [x_agg[c * BPC:(c + 1) * BPC].T,
             tod_emb[tod_idx[c * BPC:(c + 1) * BPC]].T], axis=0)
        in_maps.append({
            "xg": np.ascontiguousarray(xg).astype(NPBF),
            "negbT": np.ascontiguousarray(negbT).astype(NPBF),
            "qm01": np.ascontiguousarray(qm01),
            "S": S.astype(NPBF),
            "tail": np.ascontiguousarray(tail).astype(NPBF),
        })
    return in_maps


def kernel(_trace=False, **inputs):
    wts = _prep_weights(inputs)
    nlong = wts["_nlong"]
    nc = build_nc(wts)
    if not nc.is_finalized():
        nc.finalize()
    in_maps = make_in_maps(inputs, nlong)
    res = run_bass_kernel_spmd(nc, in_maps, core_ids=list(range(NCORES)),
                               trace=_trace)
    out = np.zeros((B, DOUT), np.float32)
    for c in range(NCORES):
        out[c * BPC:(c + 1) * BPC, :] = res.results[c]["outT"].T
    if _trace:
        kernel._last_results = res
    return out


# revision 4
# speedup vs baseline: 1.0351x; 1.0351x over previous
"""Trainium2 Bass kernel for nn_DeliveryEventEncoder — v2 "packed tiles".

Data-parallel over 8 cores (128 units/core). Per core, units are sorted by
length and tile-quantized: unit u owns kt=ceil(len/128) in {1,2} token tiles;
only those TT = sum(kt) ~ 1.5*NU tiles (vs 2*NU dense) are processed. The
module is specialized on the observed lengths (rebuilt per kernel() call);
all 8 cores share one SPMD module sized to the max long-count across cores.

Key structural choices vs the per-unit baseline:
- Units are grouped into chunks of <=4 tiles; all elementwise work runs on
  [128, 512]-wide tiles (one PSUM bank), amortizing fixed engine latencies.
- Two mega-phases split by ACT table: phase 1 (attention, exp) for all
  chunks, then phase 2 (LayerNorm sqrt, FFN) -> exactly 2 table loads.
- softmax reciprocal eliminated: LN(emb + ao/den) == LN(den*emb + ao) by
  LayerNorm scale invariance, so the denominator multiplies the embedding
  (per-partition scalar in token-major layout) instead of dividing ao.
- key masking is folded into the exp: a rank-1 PE matmul adds -400 to
  invalid-key score rows, so exp gives exact zeros; query masking folds
  into LN2's rstd (batched, one op per supergroup).
- LN stats come for free from accum_out on the residual-add/square ops and
  are post-processed in a few [128, TT]-wide batched ops.
"""

import os
import numpy as np
import ml_dtypes

import concourse.bass as bass
import concourse.bacc as bacc_mod
import concourse.mybir as mybir
import concourse.tile as tile
from concourse.bass_utils import run_bass_kernel_spmd
from concourse.masks import make_identity

F32 = mybir.dt.float32
BF16 = mybir.dt.bfloat16
AF = mybir.ActivationFunctionType
ALU = mybir.AluOpType
NPBF = ml_dtypes.bfloat16

B, U, L, DSEQ, H, DOUT = 32, 32, 256, 5, 128, 128
TODV, TODD, AGGD, UNITD = 5, 3, 7, 16
NCORES = 8
BPC = B // NCORES          # buildings per core
NU = BPC * U               # units per core (128)
CSCALE = 1.0 / np.sqrt(H)
EPS = 1e-5
NEGB = -400.0              # pre-scale score bias for invalid keys


def _chunks_of(kt):
    """Greedy chunks of units with sum(kt) <= 4 tiles each."""
    chunks, cur, cnt = [], [], 0
    for u in range(len(kt)):
        if cnt + kt[u] > 4:
            chunks.append(cur)
            cur, cnt = [], 0
        cur.append(u)
        cnt += kt[u]
    if cur:
        chunks.append(cur)
    return chunks


def build_nc(wts):
    wts = dict(wts)
    nlong = int(wts.pop("_nlong"))
    kt = [1] * (NU - nlong) + [2] * nlong
    tstart = np.concatenate([[0], np.cumsum(kt)]).astype(int)  # tile index per unit
    TT = int(tstart[NU])
    chunks = _chunks_of(kt)
    NCH = len(chunks)

    SGC = int(os.environ.get("KSGC", "16"))
    routes = dict(embT="dve", yT="act", vs="act", en="mm", enc="dve",
                  stt1="dve", sq1="dve", norm1="dve", x1t="dma",
                  x1tc="dve", relu="act", x2c="act", sq2="dve")
    for kv in os.environ.get("KROUTE", "").split(","):
        if kv:
            k_, v_ = kv.split("=")
            routes[k_] = v_

    nc = bacc_mod.Bacc()

    x_in = nc.dram_tensor("xg", [DSEQ, TT * 128], BF16, kind="ExternalInput")
    NB3 = (TT + 2) // 3
    negb_in = nc.dram_tensor("negbT", [65, NB3 * 128], BF16, kind="ExternalInput")
    qm_in = nc.dram_tensor("qm01", [128, 256], F32, kind="ExternalInput")
    s_in = nc.dram_tensor("S", [NU, BPC], BF16, kind="ExternalInput")
    tail_in = nc.dram_tensor("tail", [AGGD + TODD, BPC], BF16, kind="ExternalInput")
    out_t = nc.dram_tensor("outT", [DOUT, BPC], F32, kind="ExternalOutput")

    dW = {k: nc.inline_tensor(v, name=k) for k, v in wts.items()}

    with tile.TileContext(nc) as tc:
        with (
            tc.tile_pool(name="singles", bufs=1) as singles,
            tc.tile_pool(name="xpool", bufs=4) as xpool,
            tc.tile_pool(name="cp", bufs=int(os.environ.get("KCP", "10"))) as cp,
            tc.tile_pool(name="esp", bufs=int(os.environ.get("KESP", "10"))) as esp,
            tc.tile_pool(name="wk2", bufs=int(os.environ.get("KWK", "10"))) as wk2,
            tc.tile_pool(name="pA", bufs=int(os.environ.get("KPA", "3")), space="PSUM") as pA,
            tc.tile_pool(name="pB", bufs=int(os.environ.get("KPB", "3")), space="PSUM") as pB,
            tc.tile_pool(name="pTT", bufs=1, space="PSUM") as pTT,
            tc.tile_pool(name="pS", bufs=2, space="PSUM") as pS,    # small (den/pool)
        ):
            # ---- constants ----
            def load_w(name, p, f):
                t = singles.tile([p, f], BF16, tag=name)
                nc.sync.dma_start(out=t, in_=dW[name][:, :])
                return t

            w_in = load_w("w_inT", DSEQ, H)
            w_g = load_w("w_gT", H, H)
            w_vo = load_w("w_voT", H, H)
            w_f1 = load_w("w_f1T", H, H)
            w_f2 = load_w("w_f2T", H, H)
            c2col = load_w("w_c2col", H, 1)
            w_u = load_w("w_uT", H, UNITD)
            w_c1 = load_w("w_c1T", UNITD + AGGD + TODD, H)
            w_c2 = load_w("w_c2T", H, DOUT)

            identb = singles.tile([128, 128], BF16, tag="identb")
            make_identity(nc, identb)
            identf = singles.tile([128, 128], F32, tag="identf")
            make_identity(nc, identf)
            ones_col = singles.tile([128, 1], BF16, tag="ones_col")
            nc.vector.memset(ones_col, 1.0)
            ones_mat = singles.tile([128, 128], BF16, tag="ones_mat")
            nc.vector.memset(ones_mat, 1.0)
            ones_row = singles.tile([65, 512], BF16, tag="ones_row")
            nc.vector.memset(ones_row, 1.0)
            eps_col = singles.tile([128, 1], F32, tag="eps")
            nc.vector.memset(eps_col, EPS)

            negb_sb = singles.tile([65, NB3 * 128], BF16, tag="negb")
            nc.sync.dma_start(out=negb_sb, in_=negb_in[:, :])
            qm_sb = singles.tile([128, 256], F32, tag="qm")
            nc.sync.dma_start(out=qm_sb, in_=qm_in[:, :])
            s_sb = singles.tile([NU, BPC], BF16, tag="S")
            nc.sync.dma_start(out=s_sb, in_=s_in[:, :])

            # big persistent activations / stats
            x1in_all = singles.tile([128, TT * 128], BF16, tag="x1in")
            x2in_sg = singles.tile([128, SGC * 512], BF16, tag="x2in")
            s1g = singles.tile([128, 256], F32, tag="s1g")
            q1g = singles.tile([128, 256], F32, tag="q1g")
            s2g = singles.tile([128, 256], F32, tag="s2g")
            q2g = singles.tile([128, 256], F32, tag="q2g")
            mean1 = singles.tile([128, 256], F32, tag="mean1")
            rstd1 = singles.tile([128, 256], F32, tag="rstd1")
            mb1 = singles.tile([128, 256], F32, tag="mb1")
            mean2 = singles.tile([128, 256], F32, tag="mean2")
            r2mb = singles.tile([128, 256], BF16, tag="r2mb")
            nmrm = singles.tile([128, 256], BF16, tag="nmrm")
            rstd2m = singles.tile([128, 256], F32, tag="rstd2m")
            tmpa = singles.tile([128, 256], F32, tag="tmpa")
            tmpb = singles.tile([128, 256], F32, tag="tmpb")
            junk = singles.tile([128, 512], BF16, tag="junk")
            pooled = singles.tile([H, NU], BF16, tag="pooled")

            mm = nc.tensor.matmul

            # ---- engine routing (tunable via KROUTE env) ----
            ENG = {"dve": nc.vector, "pool": nc.gpsimd}

            def rt_copy(kind, out, in_):
                r = routes[kind]
                if r == "act":
                    nc.scalar.activation(out=out, in_=in_, func=AF.Copy,
                                         bias=0.0, scale=1.0)
                else:
                    ENG[r].tensor_copy(out, in_)

            def rt_stt(kind, **kw):
                r = routes[kind]
                if r == "act" and kw.get("op1") == ALU.mult:
                    nc.scalar.activation(out=kw["out"], in_=kw["in0"],
                                         func=AF.Square, bias=0.0, scale=1.0,
                                         accum_out=kw["accum_out"])
                else:
                    ENG[r].scalar_tensor_tensor(**kw)

            def rt_norm(kind, out, in_, mean_c, rstd_c, mbias_c):
                r = routes[kind]
                if r == "act":
                    nc.scalar.activation(out=out, in_=in_, func=AF.Identity,
                                         bias=mbias_c, scale=rstd_c)
                else:
                    ENG[r].tensor_scalar(out=out, in0=in_, scalar1=mean_c,
                                         scalar2=rstd_c, op0=ALU.subtract,
                                         op1=ALU.mult)

            def rt_relu(kind, out, in_):
                r = routes[kind]
                if r == "act":
                    nc.scalar.activation(out=out, in_=in_, func=AF.Relu,
                                         bias=0.0, scale=1.0)
                else:
                    ENG[r].tensor_scalar_max(out=out, in0=in_, scalar1=0.0)

            # chunk geometry helpers
            def cgeom(ch):
                """units with local tile offsets; returns (units, w) where
                units = [(u, base_tile_global, base_local)] and w = #tiles."""
                units = []
                loc = 0
                for u in ch:
                    units.append((u, int(tstart[u]), loc))
                    loc += kt[u]
                return units, loc

            def boundary(u, j):
                """tile j of unit u is its last tile (may be ragged)."""
                return j == kt[u] - 1

            def negb_row(t):
                p = 32 * (t % 3)
                c = (t // 3) * 128
                return negb_sb[p:p + 1, c:c + 128]

            # ================= phase 1 chunk body (exp table) =================
            def emit_p1(ch):
                units, w = cgeom(ch)
                W = w * 128
                t0 = int(tstart[ch[0]])

                xs = xpool.tile([DSEQ, 512], BF16, tag="xs")
                nc.sync.dma_start(out=xs[:, :W], in_=x_in[:, t0 * 128:(t0 + w) * 128])

                emb_ps = pA.tile([128, 512], F32, tag="pA")
                mm(emb_ps[:, :W], w_in, xs[:, :W], start=True, stop=True)
                embT = cp.tile([128, 512], BF16, tag="embT")
                rt_copy("embT", embT[:, :W], emb_ps[:, :W])

                if routes["en"] == "dma":
                    en_sb = cp.tile([128, 512], BF16, tag="en")
                    nc.sync.dma_start_transpose(
                        en_sb[:, :W].rearrange("t (b h) -> t b h", b=w),
                        embT[:, :W])
                else:
                    en_ps = pB.tile([128, 512], F32, tag="pB")
                    for j in range(w):
                        mm(en_ps[:, j * 128:(j + 1) * 128],
                           xs[:, j * 128:(j + 1) * 128], w_in,
                           start=True, stop=True)
                    en_sb = cp.tile([128, 512], BF16, tag="en")
                    rt_copy("enc", en_sb[:, :W], en_ps[:, :W])

                y_ps = pA.tile([128, 512], F32, tag="pA")
                mm(y_ps[:, :W], w_g, embT[:, :W], start=True, stop=True)
                yT = cp.tile([128, 512], BF16, tag="yT")
                rt_copy("yT", yT[:, :W], y_ps[:, :W])

                v_ps = pA.tile([128, 512], F32, tag="pA")
                for j in range(w):
                    mm(v_ps[:, j * 128:(j + 1) * 128],
                       embT[:, j * 128:(j + 1) * 128], w_vo, start=True, stop=True)
                vs = cp.tile([128, 512], BF16, tag="vs")
                rt_copy("vs", vs[:, :W], v_ps[:, :W])

                # scores + exp, one bank per key-tile index (mt)
                maxkt = max(kt[u] for u in ch)
                es_mt = []
                for mt in range(maxkt):
                    # column layout within this mt-bank: q-ranges of units with kt>mt
                    sc_ps = pA.tile([128, 512], F32, tag="pA")
                    col = 0
                    spans = []  # (u, base_local, qw, col)
                    for (u, bg, bl) in units:
                        if kt[u] <= mt:
                            continue
                        qw = kt[u] * 128
                        lhs = embT[:, (bl + mt) * 128:(bl + mt + 1) * 128]
                        bnd = boundary(u, mt)
                        mm(sc_ps[:, col:col + qw], lhs,
                           yT[:, bl * 128:bl * 128 + qw],
                           start=True,
                           stop=(bnd and bool(os.environ.get("KNONEGB"))) or not bnd)
                        if bnd and not os.environ.get("KNONEGB"):
                            p3 = 32 * ((bg + mt) % 3)
                            mm(sc_ps[:, col:col + qw], negb_row(bg + mt),
                               ones_row[p3:p3 + 1, :qw], start=False, stop=True)
                        spans.append((u, bl, qw, col))
                        col += qw
                    es = esp.tile([128, 512], BF16, tag=f"es{mt}")
                    nc.scalar.activation(out=es[:, :col], in_=sc_ps[:, :col],
                                         func=AF.Exp, bias=0.0, scale=CSCALE)
                    es_mt.append((es, spans))

                # fused: den (exp row-sums) and ao_proj = es.T @ (emb @ Wv Wo)
                # share the same stationary es tile per (unit, qtile, ktile)
                den_ps = pS.tile([128, 16], F32, tag="sm")
                pon_ps = pB.tile([128, 512], F32, tag="pB")
                for (u, bg, bl) in units:
                    for j in range(kt[u]):
                        for mt in range(kt[u]):
                            es, spans = es_mt[mt]
                            ucol = next(c for (uu, _, _, c) in spans if uu == u)
                            lhs = es[:, ucol + j * 128:ucol + (j + 1) * 128]
                            st, sp = (mt == 0), (mt == kt[u] - 1)
                            mm(den_ps[:, bl + j:bl + j + 1], lhs, ones_col,
                               start=st, stop=sp)
                            mm(pon_ps[:, (bl + j) * 128:(bl + j + 1) * 128],
                               lhs, vs[:, (bl + mt) * 128:(bl + mt + 1) * 128],
                               start=st, stop=sp)

                if os.environ.get("KDENSB", "0") == "1":
                    den_sb = cp.tile([128, 16], F32, tag="den_sb")
                    nc.vector.tensor_copy(den_sb[:, :w], den_ps[:, :w])
                else:
                    den_sb = den_ps
                # x1in = den*emb + ao_proj  (token-major), stats via accum
                for j in range(w):
                    t = t0 + j
                    xsl = x1in_all[:, t * 128:(t + 1) * 128]
                    rt_stt("stt1", out=xsl, in0=en_sb[:, j * 128:(j + 1) * 128],
                           scalar=den_sb[:, j:j + 1],
                           in1=pon_ps[:, j * 128:(j + 1) * 128],
                           op0=ALU.mult, op1=ALU.add,
                           accum_out=s1g[:, t:t + 1])
                    rt_stt("sq1", out=junk[:, :128], in0=xsl, scalar=1.0, in1=xsl,
                           op0=ALU.mult, op1=ALU.mult,
                           accum_out=q1g[:, t:t + 1])

            # ---- batched LN1 stats: rstd1, mb1 = -mean*rstd ----
            def emit_stats1(ta, tb):
                nc.vector.tensor_scalar(out=mean1[:, ta:tb], in0=s1g[:, ta:tb],
                                        scalar1=1.0 / H, scalar2=None, op0=ALU.mult)
                nc.vector.tensor_tensor(out=tmpa[:, ta:tb], in0=mean1[:, ta:tb],
                                        in1=mean1[:, ta:tb], op=ALU.mult)
                nc.vector.scalar_tensor_tensor(
                    out=tmpb[:, ta:tb], in0=q1g[:, ta:tb], scalar=1.0 / H,
                    in1=tmpa[:, ta:tb], op0=ALU.mult, op1=ALU.subtract)
                nc.scalar.activation(out=tmpa[:, ta:tb], in_=tmpb[:, ta:tb],
                                     func=AF.Sqrt, bias=eps_col, scale=1.0)
                nc.vector.reciprocal(rstd1[:, ta:tb], tmpa[:, ta:tb])
                nc.vector.scalar_tensor_tensor(
                    out=mb1[:, ta:tb], in0=mean1[:, ta:tb], scalar=-1.0,
                    in1=rstd1[:, ta:tb], op0=ALU.mult, op1=ALU.mult)

            # ================= phase 2 supergroup (sqrt table) =================
            def emit_p2(sg0):
                sgch = chunks[sg0:sg0 + SGC]
                # B2: normalize, FFN, residual + stats
                for si, ch in enumerate(sgch):
                    units, w = cgeom(ch)
                    W = w * 128
                    t0 = int(tstart[ch[0]])
                    x1 = wk2.tile([128, 512], BF16, tag="x1")
                    for j in range(w):
                        t = t0 + j
                        rt_norm("norm1", x1[:, j * 128:(j + 1) * 128],
                                x1in_all[:, t * 128:(t + 1) * 128],
                                mean1[:, t:t + 1], rstd1[:, t:t + 1],
                                mb1[:, t:t + 1])
                    x1T = wk2.tile([128, 512], BF16, tag="x1T")
                    if routes.get("x1t", "dma") == "dma":
                        nc.sync.dma_start_transpose(
                            x1T[:, :W].rearrange("h (b t) -> h b t", b=w),
                            x1[:, :W])
                    else:
                        x1t_ps = pTT.tile([128, 512], BF16, tag="pBt")
                        for j in range(w):
                            nc.tensor.transpose(
                                x1t_ps[:, j * 128:(j + 1) * 128],
                                x1[:, j * 128:(j + 1) * 128], identb)
                        rt_copy("x1tc", x1T[:, :W], x1t_ps[:, :W])

                    f1_ps = pA.tile([128, 512], F32, tag="pA")
                    mm(f1_ps[:, :W], w_f1, x1T[:, :W], start=True, stop=True)
                    f1r = wk2.tile([128, 512], BF16, tag="f1r")
                    rt_relu("relu", f1r[:, :W], f1_ps[:, :W])

                    f2_ps = pB.tile([128, 512], F32, tag="pB")
                    s2_ps = pS.tile([128, 16], F32, tag="sm")
                    for j in range(w):
                        f1sl = f1r[:, j * 128:(j + 1) * 128]
                        x1sl = x1T[:, j * 128:(j + 1) * 128]
                        mm(f2_ps[:, j * 128:(j + 1) * 128], f1sl, w_f2,
                           start=True, stop=False)
                        mm(f2_ps[:, j * 128:(j + 1) * 128], x1sl, identb,
                           start=False, stop=True)
                        mm(s2_ps[:, j:j + 1], f1sl, c2col, start=True, stop=False)
                        mm(s2_ps[:, j:j + 1], x1sl, ones_col, start=False, stop=True)
                    if os.environ.get("KSMC", "act") == "act":
                        nc.scalar.activation(out=s2g[:, t0:t0 + w], in_=s2_ps[:, :w],
                                             func=AF.Copy, bias=0.0, scale=1.0)
                    else:
                        nc.vector.tensor_copy(s2g[:, t0:t0 + w], s2_ps[:, :w])
                    xsl = x2in_sg[:, si * 512:si * 512 + W]
                    rt_copy("x2c", xsl, f2_ps[:, :W])
                    for j in range(w):
                        t = t0 + j
                        xj = x2in_sg[:, (si * 4 + j) * 128:(si * 4 + j + 1) * 128]
                        rt_stt("sq2", out=junk[:, :128], in0=xj, scalar=1.0,
                               in1=xj, op0=ALU.mult, op1=ALU.mult,
                               accum_out=q2g[:, t:t + 1])

                # batched LN2 stats for this supergroup
                ta = int(tstart[sgch[0][0]])
                tb = int(tstart[sgch[-1][-1]]) + kt[sgch[-1][-1]]
                nt = tb - ta
                nc.vector.tensor_scalar(out=mean2[:, ta:tb], in0=s2g[:, ta:tb],
                                        scalar1=1.0 / H, scalar2=None, op0=ALU.mult)
                nc.vector.tensor_tensor(out=tmpa[:, ta:tb], in0=mean2[:, ta:tb],
                                        in1=mean2[:, ta:tb], op=ALU.mult)
                nc.vector.scalar_tensor_tensor(
                    out=tmpb[:, ta:tb], in0=q2g[:, ta:tb], scalar=1.0 / H,
                    in1=tmpa[:, ta:tb], op0=ALU.mult, op1=ALU.subtract)
                nc.scalar.activation(out=tmpa[:, ta:tb], in_=tmpb[:, ta:tb],
                                     func=AF.Sqrt, bias=eps_col, scale=1.0)
                nc.vector.reciprocal(tmpb[:, ta:tb], tmpa[:, ta:tb])
                nc.vector.tensor_tensor(out=rstd2m[:, ta:tb], in0=tmpb[:, ta:tb],
                                        in1=qm_sb[:, ta:tb], op=ALU.mult)
                nc.vector.tensor_copy(r2mb[:, ta:tb], rstd2m[:, ta:tb])
                nc.vector.scalar_tensor_tensor(
                    out=nmrm[:, ta:tb], in0=mean2[:, ta:tb], scalar=-1.0,
                    in1=rstd2m[:, ta:tb], op0=ALU.mult, op1=ALU.mult)

                # B3: normalize + masked sum-pool
                for si, ch in enumerate(sgch):
                    units, w = cgeom(ch)
                    t0 = int(tstart[ch[0]])
                    pool_ps = pS.tile([128, 16], F32, tag="sm")
                    for ui, (u, bg, bl) in enumerate(units):
                        for j in range(kt[u]):
                            t = t0 + bl + j
                            x2sl = x2in_sg[:, (si * 4 + bl + j) * 128:
                                           (si * 4 + bl + j + 1) * 128]
                            mm(pool_ps[:, ui:ui + 1], x2sl, r2mb[:, t:t + 1],
                               start=(j == 0), stop=False)
                            mm(pool_ps[:, ui:ui + 1], ones_mat,
                               nmrm[:, t:t + 1],
                               start=False, stop=(j == kt[u] - 1))
                    if os.environ.get("KSMC", "act") == "act":
                        nc.scalar.activation(
                            out=pooled[:, ch[0]:ch[0] + len(units)],
                            in_=pool_ps[:, :len(units)],
                            func=AF.Copy, bias=0.0, scale=1.0)
                    else:
                        nc.vector.tensor_copy(
                            pooled[:, ch[0]:ch[0] + len(units)],
                            pool_ps[:, :len(units)])

            # ================= interleaved driver =================
            def sg_trange(sg0):
                sgch = chunks[sg0:sg0 + SGC]
                ta = int(tstart[sgch[0][0]])
                tb = int(tstart[sgch[-1][-1]]) + kt[sgch[-1][-1]]
                return ta, tb

            LAG = int(os.environ.get("KLAG", "0"))
            sgs = list(range(0, NCH, SGC))
            if LAG == 0:
                for ch in chunks:
                    emit_p1(ch)
                for sg0 in sgs:
                    emit_stats1(*sg_trange(sg0))
                    emit_p2(sg0)
            else:
                for i, sg0 in enumerate(sgs):
                    for ch in chunks[sg0:sg0 + SGC]:
                        emit_p1(ch)
                    if i >= LAG:
                        prev = sgs[i - LAG]
                        emit_stats1(*sg_trange(prev))
                        emit_p2(prev)
                for i in range(max(0, len(sgs) - LAG), len(sgs)):
                    emit_stats1(*sg_trange(sgs[i]))
                    emit_p2(sgs[i])

            # ================= tail: unit_fc, building sum, fusion =================
            u16_ps = pB.tile([UNITD, NU], F32, tag="pB")
            mm(u16_ps, w_u, pooled, start=True, stop=True)
            u16 = cp.tile([UNITD, NU], F32, tag="u16")
            nc.scalar.activation(out=u16, in_=u16_ps, func=AF.Relu,
                                 bias=0.0, scale=1.0)

            u16t_ps = pB.tile([NU, UNITD], F32, tag="pB")
            nc.tensor.transpose(u16t_ps, u16, identf[:UNITD, :UNITD])
            u16t = cp.tile([NU, UNITD], BF16, tag="u16t")
            nc.vector.tensor_copy(u16t, u16t_ps)

            seq_ps = pB.tile([UNITD, BPC], F32, tag="pB")
            mm(seq_ps, u16t, s_sb, start=True, stop=True)

            fused = cp.tile([UNITD + AGGD + TODD, BPC], BF16, tag="fused")
            nc.vector.tensor_copy(fused[:UNITD, :], seq_ps)
            nc.sync.dma_start(out=fused[UNITD:, :], in_=tail_in[:, :])

            h1_ps = pB.tile([H, BPC], F32, tag="pB")
            mm(h1_ps, w_c1, fused, start=True, stop=True)
            h1 = cp.tile([H, BPC], BF16, tag="h1")
            nc.scalar.activation(out=h1, in_=h1_ps, func=AF.Relu,
                                 bias=0.0, scale=1.0)

            o_ps = pB.tile([DOUT, BPC], F32, tag="pB")
            mm(o_ps, w_c2, h1, start=True, stop=True)
            o_s = cp.tile([DOUT, BPC], F32, tag="osb")
            nc.scalar.activation(out=o_s, in_=o_ps, func=AF.Relu,
                                 bias=0.0, scale=1.0)
            nc.sync.dma_start(out=out_t[:, :], in_=o_s)

    return nc


def _prep_weights(inputs):
    ipw = np.asarray(inputs["in_proj_w"])
    wts = {
        "w_inT": np.asarray(inputs["W_in"]).T,        # [5,128]
        "w_gT": (ipw[0:H] @ ipw[H:2 * H].T),          # composed q/k [128,128]
        "w_voT": ipw[2 * H:3 * H].T @ np.asarray(inputs["out_proj_w"]).T,
        "w_f1T": np.asarray(inputs["W_ff1"]).T,
        "w_f2T": np.asarray(inputs["W_ff2"]).T,
        "w_c2col": np.asarray(inputs["W_ff2"]).T.sum(axis=1, keepdims=True),
        "w_uT": np.asarray(inputs["W_unit"]).T,       # [128,16]
        "w_c1T": np.asarray(inputs["W_fc1"]).T,       # [26,128]
        "w_c2T": np.asarray(inputs["W_fc2"]).T,       # [128,128]
    }
    wts = {k: np.ascontiguousarray(v.astype(NPBF)) for k, v in wts.items()}
    for nm in ("b_in", "in_proj_b", "out_proj_b", "b_ff1", "b_ff2",
               "ln1_b", "ln2_b", "b_unit", "b_fc1", "b_fc2"):
        assert np.max(np.abs(np.asarray(inputs[nm]))) == 0.0, f"{nm} nonzero"
    for nm in ("ln1_w", "ln2_w"):
        assert np.allclose(np.asarray(inputs[nm]), 1.0), f"{nm} nontrivial"

    lengths = np.asarray(inputs["lengths"]).reshape(NCORES, NU)
    nlong = int(max((lengths[c] > 128).sum() for c in range(NCORES)))
    wts["_nlong"] = nlong
    return wts


def make_in_maps(inputs, nlong):
    x_seq = np.asarray(inputs["x_seq"], dtype=np.float32)        # [B,U,L,5]
    lengths = np.asarray(inputs["lengths"]).reshape(NCORES, NU)
    x_agg = np.asarray(inputs["x_agg_quant"], dtype=np.float32)  # [B,7]
    tod_emb = np.asarray(inputs["tod_emb"], dtype=np.float32)    # [5,3]
    tod_idx = np.asarray(inputs["tod_idx"])                      # [B]

    kt_mod = np.array([1] * (NU - nlong) + [2] * nlong)
    tstart = np.concatenate([[0], np.cumsum(kt_mod)]).astype(int)
    TT = int(tstart[NU])

    in_maps = []
    for c in range(NCORES):
        lens = lengths[c]
        xc = x_seq[c * BPC:(c + 1) * BPC].reshape(NU, L, DSEQ)
        # sort units: shorts (len<=128) first
        order = np.argsort(lens > 128, kind="stable")
        xg = np.zeros((DSEQ, TT * 128), np.float32)
        NB3 = (TT + 2) // 3
        negbT = np.zeros((65, NB3 * 128), np.float32)
        qm01 = np.zeros((128, 256), np.float32)
        for i in range(NU):
            u = order[i]
            ln = int(lens[u])
            t0 = int(tstart[i])
            ntile = int(kt_mod[i])
            for j in range(ntile):
                t = t0 + j
                lo = j * 128
                valid = max(0, min(128, ln - lo))
                if valid > 0:
                    xg[:, t * 128:t * 128 + valid] = \
                        xc[u, lo:lo + valid, :].T
                negbT[32 * (t % 3), (t // 3) * 128 + valid:(t // 3 + 1) * 128] = NEGB
                qm01[:valid, t] = 1.0
        S = np.zeros((NU, BPC), np.float32)
        S[np.arange(NU), order // U] = 1.0
        tail = np.concatenate(
            [x_agg[c * BPC:(c + 1) * BPC].T,
             tod_emb[tod_idx[c * BPC:(c + 1) * BPC]].T], axis=0)
        in_maps.append({
            "xg": np.ascontiguousarray(xg).astype(NPBF),
            "negbT": np.ascontiguousarray(negbT).astype(NPBF),
            "qm01": np.ascontiguousarray(qm01),
            "S": S.astype(NPBF),
            "tail": np.ascontiguousarray(tail).astype(NPBF),
        })
    return in_maps


def kernel(_trace=False, **inputs):
    wts = _prep_weights(inputs)
    nlong = wts["_nlong"]
    nc = build_nc(wts)
    if not nc.is_finalized():
        nc.finalize()
    in_maps = make_in_maps(inputs, nlong)
    res = run_bass_kernel_spmd(nc, in_maps, core_ids=list(range(NCORES)),
                               trace=_trace)
    out = np.zeros((B, DOUT), np.float32)
    for c in range(NCORES):
        out[c * BPC:(c + 1) * BPC, :] = res.results[c]["outT"].T
    if _trace:
        kernel._last_results = res
    return out


# revision 5
# speedup vs baseline: 1.0427x; 1.0074x over previous
"""Trainium2 Bass kernel for nn_DeliveryEventEncoder — v2 "packed tiles".

Data-parallel over 8 cores (128 units/core). Per core, units are sorted by
length and tile-quantized: unit u owns kt=ceil(len/128) in {1,2} token tiles;
only those TT = sum(kt) ~ 1.5*NU tiles (vs 2*NU dense) are processed. The
module is specialized on the observed lengths (rebuilt per kernel() call);
all 8 cores share one SPMD module sized to the max long-count across cores.

Key structural choices vs the per-unit baseline:
- Units are grouped into chunks of <=4 tiles; all elementwise work runs on
  [128, 512]-wide tiles (one PSUM bank), amortizing fixed engine latencies.
- Two mega-phases split by ACT table: phase 1 (attention, exp) for all
  chunks, then phase 2 (LayerNorm sqrt, FFN) -> exactly 2 table loads.
- softmax reciprocal eliminated: LN(emb + ao/den) == LN(den*emb + ao) by
  LayerNorm scale invariance, so the denominator multiplies the embedding
  (per-partition scalar in token-major layout) instead of dividing ao.
- key masking is folded into the exp: a rank-1 PE matmul adds -400 to
  invalid-key score rows, so exp gives exact zeros; query masking folds
  into LN2's rstd (batched, one op per supergroup).
- LN stats come for free from accum_out on the residual-add/square ops and
  are post-processed in a few [128, TT]-wide batched ops.
"""

import os
import numpy as np
import ml_dtypes

import concourse.bass as bass
import concourse.bacc as bacc_mod
import concourse.mybir as mybir
import concourse.tile as tile
from concourse.bass_utils import run_bass_kernel_spmd
from concourse.masks import make_identity

F32 = mybir.dt.float32
BF16 = mybir.dt.bfloat16
AF = mybir.ActivationFunctionType
ALU = mybir.AluOpType
NPBF = ml_dtypes.bfloat16

B, U, L, DSEQ, H, DOUT = 32, 32, 256, 5, 128, 128
TODV, TODD, AGGD, UNITD = 5, 3, 7, 16
NCORES = 8
BPC = B // NCORES          # buildings per core
NU = BPC * U               # units per core (128)
CSCALE = 1.0 / np.sqrt(H)
EPS = 1e-5
NEGB = -400.0              # pre-scale score bias for invalid keys


def _chunks_of(kt):
    """Greedy chunks of units with sum(kt) <= 4 tiles each."""
    chunks, cur, cnt = [], [], 0
    for u in range(len(kt)):
        if cnt + kt[u] > 4:
            chunks.append(cur)
            cur, cnt = [], 0
        cur.append(u)
        cnt += kt[u]
    if cur:
        chunks.append(cur)
    return chunks


def build_nc(wts):
    wts = dict(wts)
    nlong = int(wts.pop("_nlong"))
    kt = [1] * (NU - nlong) + [2] * nlong
    tstart = np.concatenate([[0], np.cumsum(kt)]).astype(int)  # tile index per unit
    TT = int(tstart[NU])
    chunks = _chunks_of(kt)
    NCH = len(chunks)

    SGC = int(os.environ.get("KSGC", "16"))
    routes = dict(embT="dve", yT="act", vs="act", en="mm", enc="dve",
                  stt1="dve", sq1="dve", norm1="dve", x1t="dma",
                  x1tc="dve", relu="act", x2c="act", sq2="dve")
    for kv in os.environ.get("KROUTE", "").split(","):
        if kv:
            k_, v_ = kv.split("=")
            routes[k_] = v_

    nc = bacc_mod.Bacc()

    x_in = nc.dram_tensor("xg", [DSEQ, TT * 128], BF16, kind="ExternalInput")
    NB3 = (TT + 2) // 3
    negb_in = nc.dram_tensor("negbT", [65, NB3 * 128], BF16, kind="ExternalInput")
    qm_in = nc.dram_tensor("qm01", [128, 256], F32, kind="ExternalInput")
    s_in = nc.dram_tensor("S", [NU, BPC], BF16, kind="ExternalInput")
    tail_in = nc.dram_tensor("tail", [AGGD + TODD, BPC], BF16, kind="ExternalInput")
    out_t = nc.dram_tensor("outT", [DOUT, BPC], F32, kind="ExternalOutput")

    dW = {k: nc.inline_tensor(v, name=k) for k, v in wts.items()}

    with tile.TileContext(nc) as tc:
        with (
            tc.tile_pool(name="singles", bufs=1) as singles,
            tc.tile_pool(name="xpool", bufs=4) as xpool,
            tc.tile_pool(name="cp", bufs=int(os.environ.get("KCP", "10"))) as cp,
            tc.tile_pool(name="esp", bufs=int(os.environ.get("KESP", "10"))) as esp,
            tc.tile_pool(name="wk2", bufs=int(os.environ.get("KWK", "10"))) as wk2,
            tc.tile_pool(name="pA", bufs=int(os.environ.get("KPA", "3")), space="PSUM") as pA,
            tc.tile_pool(name="pB", bufs=int(os.environ.get("KPB", "3")), space="PSUM") as pB,
            tc.tile_pool(name="pTT", bufs=1, space="PSUM") as pTT,
            tc.tile_pool(name="pS", bufs=2, space="PSUM") as pS,    # small (den/pool)
        ):
            # ---- constants ----
            def load_w(name, p, f):
                t = singles.tile([p, f], BF16, tag=name)
                nc.sync.dma_start(out=t, in_=dW[name][:, :])
                return t

            w_in = load_w("w_inT", DSEQ, H)
            w_g = load_w("w_gT", H, H)
            w_vo = load_w("w_voT", H, H)
            w_f1 = load_w("w_f1T", H, H)
            w_f2 = load_w("w_f2T", H, H)
            c2col = load_w("w_c2col", H, 1)
            w_u = load_w("w_uT", H, UNITD)
            w_c1 = load_w("w_c1T", UNITD + AGGD + TODD, H)
            w_c2 = load_w("w_c2T", H, DOUT)

            identb = singles.tile([128, 128], BF16, tag="identb")
            make_identity(nc, identb)
            identf = singles.tile([128, 128], F32, tag="identf")
            make_identity(nc, identf)
            ones_col = singles.tile([128, 1], BF16, tag="ones_col")
            nc.vector.memset(ones_col, 1.0)
            ones_mat = singles.tile([128, 128], BF16, tag="ones_mat")
            nc.vector.memset(ones_mat, 1.0)
            ones_row = singles.tile([65, 512], BF16, tag="ones_row")
            nc.vector.memset(ones_row, 1.0)
            eps_col = singles.tile([128, 1], F32, tag="eps")
            nc.vector.memset(eps_col, EPS)

            negb_sb = singles.tile([65, NB3 * 128], BF16, tag="negb")
            nc.sync.dma_start(out=negb_sb, in_=negb_in[:, :])
            qm_sb = singles.tile([128, 256], F32, tag="qm")
            nc.sync.dma_start(out=qm_sb, in_=qm_in[:, :])
            s_sb = singles.tile([NU, BPC], BF16, tag="S")
            nc.sync.dma_start(out=s_sb, in_=s_in[:, :])

            # big persistent activations / stats
            x1in_all = singles.tile([128, TT * 128], BF16, tag="x1in")
            x2in_sg = singles.tile([128, SGC * 512], BF16, tag="x2in")
            s1g = singles.tile([128, 256], F32, tag="s1g")
            q1g = singles.tile([128, 256], F32, tag="q1g")
            s2g = singles.tile([128, 256], F32, tag="s2g")
            q2g = singles.tile([128, 256], F32, tag="q2g")
            mean1 = singles.tile([128, 256], F32, tag="mean1")
            rstd1 = singles.tile([128, 256], F32, tag="rstd1")
            mb1 = singles.tile([128, 256], F32, tag="mb1")
            mean2 = singles.tile([128, 256], F32, tag="mean2")
            r2mb = singles.tile([128, 256], BF16, tag="r2mb")
            nmrm = singles.tile([128, 256], BF16, tag="nmrm")
            rstd2m = singles.tile([128, 256], F32, tag="rstd2m")
            tmpa = singles.tile([128, 256], F32, tag="tmpa")
            tmpb = singles.tile([128, 256], F32, tag="tmpb")
            tmpc = singles.tile([128, 256], F32, tag="tmpc")
            tmpd = singles.tile([128, 256], F32, tag="tmpd")
            junk = singles.tile([128, 512], BF16, tag="junk")
            junk2 = singles.tile([128, 512], BF16, tag="junk2")
            pooled = singles.tile([H, NU], BF16, tag="pooled")

            mm = nc.tensor.matmul

            # ---- engine routing (tunable via KROUTE env) ----
            ENG = {"dve": nc.vector, "pool": nc.gpsimd}

            def rt_copy(kind, out, in_):
                r = routes[kind]
                if r == "split":
                    n = out.shape[-1]
                    h = ((n + 255) // 256) * 128
                    nc.vector.tensor_copy(out[:, :h], in_[:, :h])
                    if n > h:
                        nc.scalar.activation(out=out[:, h:], in_=in_[:, h:],
                                             func=AF.Copy, bias=0.0, scale=1.0)
                elif r == "act":
                    nc.scalar.activation(out=out, in_=in_, func=AF.Copy,
                                         bias=0.0, scale=1.0)
                else:
                    ENG[r].tensor_copy(out, in_)

            def rt_stt(kind, **kw):
                r = routes[kind]
                if r == "act" and kw.get("op1") == ALU.mult:
                    nc.scalar.activation(out=kw["out"], in_=kw["in0"],
                                         func=AF.Square, bias=0.0, scale=1.0,
                                         accum_out=kw["accum_out"])
                else:
                    ENG[r].scalar_tensor_tensor(**kw)

            def rt_norm(kind, out, in_, mean_c, rstd_c, mbias_c):
                r = routes[kind]
                if r == "act":
                    nc.scalar.activation(out=out, in_=in_, func=AF.Identity,
                                         bias=mbias_c, scale=rstd_c)
                else:
                    ENG[r].tensor_scalar(out=out, in0=in_, scalar1=mean_c,
                                         scalar2=rstd_c, op0=ALU.subtract,
                                         op1=ALU.mult)

            def rt_relu(kind, out, in_):
                r = routes[kind]
                if r == "act":
                    nc.scalar.activation(out=out, in_=in_, func=AF.Relu,
                                         bias=0.0, scale=1.0)
                else:
                    ENG[r].tensor_scalar_max(out=out, in0=in_, scalar1=0.0)

            # chunk geometry helpers
            def cgeom(ch):
                """units with local tile offsets; returns (units, w) where
                units = [(u, base_tile_global, base_local)] and w = #tiles."""
                units = []
                loc = 0
                for u in ch:
                    units.append((u, int(tstart[u]), loc))
                    loc += kt[u]
                return units, loc

            def boundary(u, j):
                """tile j of unit u is its last tile (may be ragged)."""
                return j == kt[u] - 1

            def negb_row(t):
                p = 32 * (t % 3)
                c = (t // 3) * 128
                return negb_sb[p:p + 1, c:c + 128]

            # ================= phase 1 chunk body (exp table) =================
            def emit_p1(ch):
                units, w = cgeom(ch)
                W = w * 128
                t0 = int(tstart[ch[0]])

                xs = xpool.tile([DSEQ, 512], BF16, tag="xs")
                nc.sync.dma_start(out=xs[:, :W], in_=x_in[:, t0 * 128:(t0 + w) * 128])

                emb_ps = pA.tile([128, 512], F32, tag="pA")
                mm(emb_ps[:, :W], w_in, xs[:, :W], start=True, stop=True)
                embT = cp.tile([128, 512], BF16, tag="embT")
                rt_copy("embT", embT[:, :W], emb_ps[:, :W])

                if routes["en"] == "dma":
                    en_sb = cp.tile([128, 512], BF16, tag="en")
                    nc.sync.dma_start_transpose(
                        en_sb[:, :W].rearrange("t (b h) -> t b h", b=w),
                        embT[:, :W])
                else:
                    en_ps = pB.tile([128, 512], F32, tag="pB")
                    for j in range(w):
                        mm(en_ps[:, j * 128:(j + 1) * 128],
                           xs[:, j * 128:(j + 1) * 128], w_in,
                           start=True, stop=True)
                    en_sb = cp.tile([128, 512], BF16, tag="en")
                    rt_copy("enc", en_sb[:, :W], en_ps[:, :W])

                y_ps = pA.tile([128, 512], F32, tag="pA")
                mm(y_ps[:, :W], w_g, embT[:, :W], start=True, stop=True)
                yT = cp.tile([128, 512], BF16, tag="yT")
                rt_copy("yT", yT[:, :W], y_ps[:, :W])

                v_ps = pA.tile([128, 512], F32, tag="pA")
                for j in range(w):
                    mm(v_ps[:, j * 128:(j + 1) * 128],
                       embT[:, j * 128:(j + 1) * 128], w_vo, start=True, stop=True)
                vs = cp.tile([128, 512], BF16, tag="vs")
                rt_copy("vs", vs[:, :W], v_ps[:, :W])

                # scores + exp, one bank per key-tile index (mt)
                maxkt = max(kt[u] for u in ch)
                es_mt = []
                for mt in range(maxkt):
                    # column layout within this mt-bank: q-ranges of units with kt>mt
                    sc_ps = pA.tile([128, 512], F32, tag="pA")
                    col = 0
                    spans = []  # (u, base_local, qw, col)
                    for (u, bg, bl) in units:
                        if kt[u] <= mt:
                            continue
                        qw = kt[u] * 128
                        lhs = embT[:, (bl + mt) * 128:(bl + mt + 1) * 128]
                        bnd = boundary(u, mt)
                        mm(sc_ps[:, col:col + qw], lhs,
                           yT[:, bl * 128:bl * 128 + qw],
                           start=True,
                           stop=(bnd and bool(os.environ.get("KNONEGB"))) or not bnd)
                        if bnd and not os.environ.get("KNONEGB"):
                            p3 = 32 * ((bg + mt) % 3)
                            mm(sc_ps[:, col:col + qw], negb_row(bg + mt),
                               ones_row[p3:p3 + 1, :qw], start=False, stop=True)
                        spans.append((u, bl, qw, col))
                        col += qw
                    es = esp.tile([128, 512], BF16, tag=f"es{mt}")
                    nc.scalar.activation(out=es[:, :col], in_=sc_ps[:, :col],
                                         func=AF.Exp, bias=0.0, scale=CSCALE)
                    es_mt.append((es, spans))

                # fused: den (exp row-sums) and ao_proj = es.T @ (emb @ Wv Wo)
                # share the same stationary es tile per (unit, qtile, ktile)
                den_ps = pS.tile([128, 16], F32, tag="sm")
                pon_ps = pB.tile([128, 512], F32, tag="pB")
                for (u, bg, bl) in units:
                    for j in range(kt[u]):
                        for mt in range(kt[u]):
                            es, spans = es_mt[mt]
                            ucol = next(c for (uu, _, _, c) in spans if uu == u)
                            lhs = es[:, ucol + j * 128:ucol + (j + 1) * 128]
                            st, sp = (mt == 0), (mt == kt[u] - 1)
                            mm(den_ps[:, bl + j:bl + j + 1], lhs, ones_col,
                               start=st, stop=sp)
                            mm(pon_ps[:, (bl + j) * 128:(bl + j + 1) * 128],
                               lhs, vs[:, (bl + mt) * 128:(bl + mt + 1) * 128],
                               start=st, stop=sp)

                if os.environ.get("KDENSB", "0") == "1":
                    den_sb = cp.tile([128, 16], F32, tag="den_sb")
                    nc.vector.tensor_copy(den_sb[:, :w], den_ps[:, :w])
                else:
                    den_sb = den_ps
                # x1in = den*emb + ao_proj  (token-major), stats via accum
                for j in range(w):
                    t = t0 + j
                    xsl = x1in_all[:, t * 128:(t + 1) * 128]
                    rt_stt("stt1", out=xsl, in0=en_sb[:, j * 128:(j + 1) * 128],
                           scalar=den_sb[:, j:j + 1],
                           in1=pon_ps[:, j * 128:(j + 1) * 128],
                           op0=ALU.mult, op1=ALU.add,
                           accum_out=s1g[:, t:t + 1])
                    rt_stt("sq1", out=junk[:, (t % 4) * 128:(t % 4 + 1) * 128],
                           in0=xsl, scalar=1.0, in1=xsl,
                           op0=ALU.mult, op1=ALU.mult,
                           accum_out=q1g[:, t:t + 1])

            # ---- batched LN1 stats: rstd1, mb1 = -mean*rstd ----
            def emit_stats1(ta, tb):
                nc.vector.tensor_scalar(out=mean1[:, ta:tb], in0=s1g[:, ta:tb],
                                        scalar1=1.0 / H, scalar2=None, op0=ALU.mult)
                nc.vector.tensor_tensor(out=tmpa[:, ta:tb], in0=mean1[:, ta:tb],
                                        in1=mean1[:, ta:tb], op=ALU.mult)
                nc.vector.scalar_tensor_tensor(
                    out=tmpb[:, ta:tb], in0=q1g[:, ta:tb], scalar=1.0 / H,
                    in1=tmpa[:, ta:tb], op0=ALU.mult, op1=ALU.subtract)
                nc.scalar.activation(out=tmpa[:, ta:tb], in_=tmpb[:, ta:tb],
                                     func=AF.Sqrt, bias=eps_col, scale=1.0)
                nc.vector.reciprocal(rstd1[:, ta:tb], tmpa[:, ta:tb])
                nc.vector.scalar_tensor_tensor(
                    out=mb1[:, ta:tb], in0=mean1[:, ta:tb], scalar=-1.0,
                    in1=rstd1[:, ta:tb], op0=ALU.mult, op1=ALU.mult)

            # ================= phase 2 supergroup (sqrt table) =================
            def emit_p2(sg0):
                sgch = chunks[sg0:sg0 + SGC]
                # B2: normalize, FFN, residual + stats
                for si, ch in enumerate(sgch):
                    units, w = cgeom(ch)
                    W = w * 128
                    t0 = int(tstart[ch[0]])
                    x1 = wk2.tile([128, 512], BF16, tag="x1")
                    for j in range(w):
                        t = t0 + j
                        rt_norm("norm1", x1[:, j * 128:(j + 1) * 128],
                                x1in_all[:, t * 128:(t + 1) * 128],
                                mean1[:, t:t + 1], rstd1[:, t:t + 1],
                                mb1[:, t:t + 1])
                    x1T = wk2.tile([128, 512], BF16, tag="x1T")
                    if routes.get("x1t", "dma") == "dma":
                        nc.sync.dma_start_transpose(
                            x1T[:, :W].rearrange("h (b t) -> h b t", b=w),
                            x1[:, :W])
                    else:
                        x1t_ps = pTT.tile([128, 512], BF16, tag="pBt")
                        for j in range(w):
                            nc.tensor.transpose(
                                x1t_ps[:, j * 128:(j + 1) * 128],
                                x1[:, j * 128:(j + 1) * 128], identb)
                        rt_copy("x1tc", x1T[:, :W], x1t_ps[:, :W])

                    f1_ps = pA.tile([128, 512], F32, tag="pA")
                    mm(f1_ps[:, :W], w_f1, x1T[:, :W], start=True, stop=True)
                    f1r = wk2.tile([128, 512], BF16, tag="f1r")
                    rt_relu("relu", f1r[:, :W], f1_ps[:, :W])

                    f2_ps = pB.tile([128, 512], F32, tag="pB")
                    s2_ps = pS.tile([128, 16], F32, tag="sm")
                    for j in range(w):
                        f1sl = f1r[:, j * 128:(j + 1) * 128]
                        x1sl = x1T[:, j * 128:(j + 1) * 128]
                        mm(f2_ps[:, j * 128:(j + 1) * 128], f1sl, w_f2,
                           start=True, stop=False)
                        mm(f2_ps[:, j * 128:(j + 1) * 128], x1sl, identb,
                           start=False, stop=True)
                        mm(s2_ps[:, j:j + 1], f1sl, c2col, start=True, stop=False)
                        mm(s2_ps[:, j:j + 1], x1sl, ones_col, start=False, stop=True)
                    if os.environ.get("KSMC", "act") == "act":
                        nc.scalar.activation(out=s2g[:, t0:t0 + w], in_=s2_ps[:, :w],
                                             func=AF.Copy, bias=0.0, scale=1.0)
                    else:
                        nc.vector.tensor_copy(s2g[:, t0:t0 + w], s2_ps[:, :w])
                    xsl = x2in_sg[:, si * 512:si * 512 + W]
                    rt_copy("x2c", xsl, f2_ps[:, :W])
                    for j in range(w):
                        t = t0 + j
                        xj = x2in_sg[:, (si * 4 + j) * 128:(si * 4 + j + 1) * 128]
                        rt_stt("sq2", out=junk2[:, (t % 4) * 128:(t % 4 + 1) * 128],
                               in0=xj, scalar=1.0,
                               in1=xj, op0=ALU.mult, op1=ALU.mult,
                               accum_out=q2g[:, t:t + 1])

                # batched LN2 stats for this supergroup
                ta = int(tstart[sgch[0][0]])
                tb = int(tstart[sgch[-1][-1]]) + kt[sgch[-1][-1]]
                nt = tb - ta
                nc.vector.tensor_scalar(out=mean2[:, ta:tb], in0=s2g[:, ta:tb],
                                        scalar1=1.0 / H, scalar2=None, op0=ALU.mult)
                nc.vector.tensor_tensor(out=tmpc[:, ta:tb], in0=mean2[:, ta:tb],
                                        in1=mean2[:, ta:tb], op=ALU.mult)
                nc.vector.scalar_tensor_tensor(
                    out=tmpd[:, ta:tb], in0=q2g[:, ta:tb], scalar=1.0 / H,
                    in1=tmpc[:, ta:tb], op0=ALU.mult, op1=ALU.subtract)
                nc.scalar.activation(out=tmpc[:, ta:tb], in_=tmpd[:, ta:tb],
                                     func=AF.Sqrt, bias=eps_col, scale=1.0)
                nc.vector.reciprocal(tmpd[:, ta:tb], tmpc[:, ta:tb])
                nc.vector.tensor_tensor(out=rstd2m[:, ta:tb], in0=tmpd[:, ta:tb],
                                        in1=qm_sb[:, ta:tb], op=ALU.mult)
                nc.vector.tensor_copy(r2mb[:, ta:tb], rstd2m[:, ta:tb])
                nc.vector.scalar_tensor_tensor(
                    out=nmrm[:, ta:tb], in0=mean2[:, ta:tb], scalar=-1.0,
                    in1=rstd2m[:, ta:tb], op0=ALU.mult, op1=ALU.mult)

                # B3: normalize + masked sum-pool
                for si, ch in enumerate(sgch):
                    units, w = cgeom(ch)
                    t0 = int(tstart[ch[0]])
                    pool_ps = pS.tile([128, 16], F32, tag="sm")
                    for ui, (u, bg, bl) in enumerate(units):
                        for j in range(kt[u]):
                            t = t0 + bl + j
                            x2sl = x2in_sg[:, (si * 4 + bl + j) * 128:
                                           (si * 4 + bl + j + 1) * 128]
                            mm(pool_ps[:, ui:ui + 1], x2sl, r2mb[:, t:t + 1],
                               start=(j == 0), stop=False)
                            mm(pool_ps[:, ui:ui + 1], ones_mat,
                               nmrm[:, t:t + 1],
                               start=False, stop=(j == kt[u] - 1))
                    if os.environ.get("KSMC", "act") == "act":
                        nc.scalar.activation(
                            out=pooled[:, ch[0]:ch[0] + len(units)],
                            in_=pool_ps[:, :len(units)],
                            func=AF.Copy, bias=0.0, scale=1.0)
                    else:
                        nc.vector.tensor_copy(
                            pooled[:, ch[0]:ch[0] + len(units)],
                            pool_ps[:, :len(units)])

            # ================= interleaved driver =================
            def sg_trange(sg0):
                sgch = chunks[sg0:sg0 + SGC]
                ta = int(tstart[sgch[0][0]])
                tb = int(tstart[sgch[-1][-1]]) + kt[sgch[-1][-1]]
                return ta, tb

            LAG = int(os.environ.get("KLAG", "0"))
            sgs = list(range(0, NCH, SGC))
            if LAG == 0:
                for ch in chunks:
                    emit_p1(ch)
                for sg0 in sgs:
                    emit_stats1(*sg_trange(sg0))
                    emit_p2(sg0)
            else:
                for i, sg0 in enumerate(sgs):
                    for ch in chunks[sg0:sg0 + SGC]:
                        emit_p1(ch)
                    if i >= LAG:
                        prev = sgs[i - LAG]
                        emit_stats1(*sg_trange(prev))
                        emit_p2(prev)
                for i in range(max(0, len(sgs) - LAG), len(sgs)):
                    emit_stats1(*sg_trange(sgs[i]))
                    emit_p2(sgs[i])

            # ================= tail: unit_fc, building sum, fusion =================
            u16_ps = pB.tile([UNITD, NU], F32, tag="pB")
            mm(u16_ps, w_u, pooled, start=True, stop=True)
            u16 = cp.tile([UNITD, NU], F32, tag="u16")
            nc.scalar.activation(out=u16, in_=u16_ps, func=AF.Relu,
                                 bias=0.0, scale=1.0)

            u16t_ps = pB.tile([NU, UNITD], F32, tag="pB")
            nc.tensor.transpose(u16t_ps, u16, identf[:UNITD, :UNITD])
            u16t = cp.tile([NU, UNITD], BF16, tag="u16t")
            nc.vector.tensor_copy(u16t, u16t_ps)

            seq_ps = pB.tile([UNITD, BPC], F32, tag="pB")
            mm(seq_ps, u16t, s_sb, start=True, stop=True)

            fused = cp.tile([UNITD + AGGD + TODD, BPC], BF16, tag="fused")
            nc.vector.tensor_copy(fused[:UNITD, :], seq_ps)
            nc.sync.dma_start(out=fused[UNITD:, :], in_=tail_in[:, :])

            h1_ps = pB.tile([H, BPC], F32, tag="pB")
            mm(h1_ps, w_c1, fused, start=True, stop=True)
            h1 = cp.tile([H, BPC], BF16, tag="h1")
            nc.scalar.activation(out=h1, in_=h1_ps, func=AF.Relu,
                                 bias=0.0, scale=1.0)

            o_ps = pB.tile([DOUT, BPC], F32, tag="pB")
            mm(o_ps, w_c2, h1, start=True, stop=True)
            o_s = cp.tile([DOUT, BPC], F32, tag="osb")
            nc.scalar.activation(out=o_s, in_=o_ps, func=AF.Relu,
                                 bias=0.0, scale=1.0)
            nc.sync.dma_start(out=out_t[:, :], in_=o_s)

    return nc


def _prep_weights(inputs):
    ipw = np.asarray(inputs["in_proj_w"])
    wts = {
        "w_inT": np.asarray(inputs["W_in"]).T,        # [5,128]
        "w_gT": (ipw[0:H] @ ipw[H:2 * H].T),          # composed q/k [128,128]
        "w_voT": ipw[2 * H:3 * H].T @ np.asarray(inputs["out_proj_w"]).T,
        "w_f1T": np.asarray(inputs["W_ff1"]).T,
        "w_f2T": np.asarray(inputs["W_ff2"]).T,
        "w_c2col": np.asarray(inputs["W_ff2"]).T.sum(axis=1, keepdims=True),
        "w_uT": np.asarray(inputs["W_unit"]).T,       # [128,16]
        "w_c1T": np.asarray(inputs["W_fc1"]).T,       # [26,128]
        "w_c2T": np.asarray(inputs["W_fc2"]).T,       # [128,128]
    }
    wts = {k: np.ascontiguousarray(v.astype(NPBF)) for k, v in wts.items()}
    for nm in ("b_in", "in_proj_b", "out_proj_b", "b_ff1", "b_ff2",
               "ln1_b", "ln2_b", "b_unit", "b_fc1", "b_fc2"):
        assert np.max(np.abs(np.asarray(inputs[nm]))) == 0.0, f"{nm} nonzero"
    for nm in ("ln1_w", "ln2_w"):
        assert np.allclose(np.asarray(inputs[nm]), 1.0), f"{nm} nontrivial"

    lengths = np.asarray(inputs["lengths"]).reshape(NCORES, NU)
    nlong = int(max((lengths[c] > 128).sum() for c in range(NCORES)))
    wts["_nlong"] = nlong
    return wts


def make_in_maps(inputs, nlong):
    x_seq = np.asarray(inputs["x_seq"], dtype=np.float32)        # [B,U,L,5]
    lengths = np.asarray(inputs["lengths"]).reshape(NCORES, NU)
    x_agg = np.asarray(inputs["x_agg_quant"], dtype=np.float32)  # [B,7]
    tod_emb = np.asarray(inputs["tod_emb"], dtype=np.float32)    # [5,3]
    tod_idx = np.asarray(inputs["tod_idx"])                      # [B]

    kt_mod = np.array([1] * (NU - nlong) + [2] * nlong)
    tstart = np.concatenate([[0], np.cumsum(kt_mod)]).astype(int)
    TT = int(tstart[NU])

    in_maps = []
    for c in range(NCORES):
        lens = lengths[c]
        xc = x_seq[c * BPC:(c + 1) * BPC].reshape(NU, L, DSEQ)
        # sort units: shorts (len<=128) first
        order = np.argsort(lens > 128, kind="stable")
        xg = np.zeros((DSEQ, TT * 128), np.float32)
        NB3 = (TT + 2) // 3
        negbT = np.zeros((65, NB3 * 128), np.float32)
        qm01 = np.zeros((128, 256), np.float32)
        for i in range(NU):
            u = order[i]
            ln = int(lens[u])
            t0 = int(tstart[i])
            ntile = int(kt_mod[i])
            for j in range(ntile):
                t = t0 + j
                lo = j * 128
                valid = max(0, min(128, ln - lo))
                if valid > 0:
                    xg[:, t * 128:t * 128 + valid] = \
                        xc[u, lo:lo + valid, :].T
                negbT[32 * (t % 3), (t // 3) * 128 + valid:(t // 3 + 1) * 128] = NEGB
                qm01[:valid, t] = 1.0
        S = np.zeros((NU, BPC), np.float32)
        S[np.arange(NU), order // U] = 1.0
        tail = np.concatenate(
            [x_agg[c * BPC:(c + 1) * BPC].T,
             tod_emb[tod_idx[c * BPC:(c + 1) * BPC]].T], axis=0)
        in_maps.append({
            "xg": np.ascontiguousarray(xg).astype(NPBF),
            "negbT": np.ascontiguousarray(negbT).astype(NPBF),
            "qm01": np.ascontiguousarray(qm01),
            "S": S.astype(NPBF),
            "tail": np.ascontiguousarray(tail).astype(NPBF),
        })
    return in_maps


def kernel(_trace=False, **inputs):
    wts = _prep_weights(inputs)
    nlong = wts["_nlong"]
    nc = build_nc(wts)
    if not nc.is_finalized():
        nc.finalize()
    in_maps = make_in_maps(inputs, nlong)
    res = run_bass_kernel_spmd(nc, in_maps, core_ids=list(range(NCORES)),
                               trace=_trace)
    out = np.zeros((B, DOUT), np.float32)
    for c in range(NCORES):
        out[c * BPC:(c + 1) * BPC, :] = res.results[c]["outT"].T
    if _trace:
        kernel._last_results = res
    return out


# revision 6
# speedup vs baseline: 1.0456x; 1.0028x over previous
"""Trainium2 Bass kernel for nn_DeliveryEventEncoder — v2 "packed tiles".

Data-parallel over 8 cores (128 units/core). Per core, units are sorted by
length and tile-quantized: unit u owns kt=ceil(len/128) in {1,2} token tiles;
only those TT = sum(kt) ~ 1.5*NU tiles (vs 2*NU dense) are processed. The
module is specialized on the observed lengths (rebuilt per kernel() call);
all 8 cores share one SPMD module sized to the max long-count across cores.

Key structural choices vs the per-unit baseline:
- Units are grouped into chunks of <=4 tiles; all elementwise work runs on
  [128, 512]-wide tiles (one PSUM bank), amortizing fixed engine latencies.
- Two mega-phases split by ACT table: phase 1 (attention, exp) for all
  chunks, then phase 2 (LayerNorm sqrt, FFN) -> exactly 2 table loads.
- softmax reciprocal eliminated: LN(emb + ao/den) == LN(den*emb + ao) by
  LayerNorm scale invariance, so the denominator multiplies the embedding
  (per-partition scalar in token-major layout) instead of dividing ao.
- key masking is folded into the exp: a rank-1 PE matmul adds -400 to
  invalid-key score rows, so exp gives exact zeros; query masking folds
  into LN2's rstd (batched, one op per supergroup).
- LN stats come for free from accum_out on the residual-add/square ops and
  are post-processed in a few [128, TT]-wide batched ops.
"""

import os
import numpy as np
import ml_dtypes

import concourse.bass as bass
import concourse.bacc as bacc_mod
import concourse.mybir as mybir
import concourse.tile as tile
from concourse.bass_utils import run_bass_kernel_spmd
from concourse.masks import make_identity

F32 = mybir.dt.float32
BF16 = mybir.dt.bfloat16
AF = mybir.ActivationFunctionType
ALU = mybir.AluOpType
NPBF = ml_dtypes.bfloat16

B, U, L, DSEQ, H, DOUT = 32, 32, 256, 5, 128, 128
TODV, TODD, AGGD, UNITD = 5, 3, 7, 16
NCORES = 8
BPC = B // NCORES          # buildings per core
NU = BPC * U               # units per core (128)
CSCALE = 1.0 / np.sqrt(H)
EPS = 1e-5
NEGB = -400.0              # pre-scale score bias for invalid keys


def _chunks_of(kt):
    """Greedy chunks of units with sum(kt) <= 4 tiles each."""
    chunks, cur, cnt = [], [], 0
    for u in range(len(kt)):
        if cnt + kt[u] > 4:
            chunks.append(cur)
            cur, cnt = [], 0
        cur.append(u)
        cnt += kt[u]
    if cur:
        chunks.append(cur)
    return chunks


def build_nc(wts):
    wts = dict(wts)
    nlong = int(wts.pop("_nlong"))
    kt = [1] * (NU - nlong) + [2] * nlong
    tstart = np.concatenate([[0], np.cumsum(kt)]).astype(int)  # tile index per unit
    TT = int(tstart[NU])
    chunks = _chunks_of(kt)
    NCH = len(chunks)

    SGC = int(os.environ.get("KSGC", "16"))
    routes = dict(embT="dve", yT="act", vs="act", en="mm", enc="dve",
                  stt1="dve", sq1="dve", norm1="dve", x1t="dma",
                  x1tc="dve", relu="act", x2c="act", sq2="dve")
    for kv in os.environ.get("KROUTE", "").split(","):
        if kv:
            k_, v_ = kv.split("=")
            routes[k_] = v_

    nc = bacc_mod.Bacc()

    x_in = nc.dram_tensor("xg", [DSEQ, TT * 128], BF16, kind="ExternalInput")
    NB3 = (TT + 2) // 3
    negb_in = nc.dram_tensor("negbT", [65, NB3 * 128], BF16, kind="ExternalInput")
    qm_in = nc.dram_tensor("qm01", [128, 256], F32, kind="ExternalInput")
    s_in = nc.dram_tensor("S", [NU, BPC], BF16, kind="ExternalInput")
    tail_in = nc.dram_tensor("tail", [AGGD + TODD, BPC], BF16, kind="ExternalInput")
    out_t = nc.dram_tensor("outT", [DOUT, BPC], F32, kind="ExternalOutput")

    dW = {k: nc.inline_tensor(v, name=k) for k, v in wts.items()}

    with tile.TileContext(nc) as tc:
        with (
            tc.tile_pool(name="singles", bufs=1) as singles,
            tc.tile_pool(name="xpool", bufs=4) as xpool,
            tc.tile_pool(name="cp", bufs=int(os.environ.get("KCP", "10"))) as cp,
            tc.tile_pool(name="esp", bufs=int(os.environ.get("KESP", "10"))) as esp,
            tc.tile_pool(name="wk2", bufs=int(os.environ.get("KWK", "10"))) as wk2,
            tc.tile_pool(name="pA", bufs=int(os.environ.get("KPA", "3")), space="PSUM") as pA,
            tc.tile_pool(name="pB", bufs=int(os.environ.get("KPB", "2")), space="PSUM") as pB,
            tc.tile_pool(name="pTT", bufs=1, space="PSUM") as pTT,
            tc.tile_pool(name="pS", bufs=int(os.environ.get("KPS", "2")), space="PSUM") as pS,    # small (den/pool)
        ):
            # ---- constants ----
            def load_w(name, p, f):
                t = singles.tile([p, f], BF16, tag=name)
                nc.sync.dma_start(out=t, in_=dW[name][:, :])
                return t

            w_in = load_w("w_inT", DSEQ, H)
            w_g = load_w("w_gT", H, H)
            w_vo = load_w("w_voT", H, H)
            w_f1 = load_w("w_f1T", H, H)
            w_f2 = load_w("w_f2T", H, H)
            c2col = load_w("w_c2col", H, 1)
            w_u = load_w("w_uT", H, UNITD)
            w_c1 = load_w("w_c1T", UNITD + AGGD + TODD, H)
            w_c2 = load_w("w_c2T", H, DOUT)

            identb = singles.tile([128, 128], BF16, tag="identb")
            make_identity(nc, identb)
            identf = singles.tile([128, 128], F32, tag="identf")
            make_identity(nc, identf)
            ones_col = singles.tile([128, 1], BF16, tag="ones_col")
            nc.vector.memset(ones_col, 1.0)
            ones_mat = singles.tile([128, 128], BF16, tag="ones_mat")
            nc.vector.memset(ones_mat, 1.0)
            ones_row = singles.tile([65, 512], BF16, tag="ones_row")
            nc.vector.memset(ones_row, 1.0)
            eps_col = singles.tile([128, 1], F32, tag="eps")
            nc.vector.memset(eps_col, EPS)

            negb_sb = singles.tile([65, NB3 * 128], BF16, tag="negb")
            nc.sync.dma_start(out=negb_sb, in_=negb_in[:, :])
            qm_sb = singles.tile([128, 256], F32, tag="qm")
            nc.sync.dma_start(out=qm_sb, in_=qm_in[:, :])
            s_sb = singles.tile([NU, BPC], BF16, tag="S")
            nc.sync.dma_start(out=s_sb, in_=s_in[:, :])

            # big persistent activations / stats
            x1in_all = singles.tile([128, TT * 128], BF16, tag="x1in")
            x2in_sg = singles.tile([128, SGC * 512], BF16, tag="x2in")
            s1g = singles.tile([128, 256], F32, tag="s1g")
            q1g = singles.tile([128, 256], F32, tag="q1g")
            s2g = singles.tile([128, 256], F32, tag="s2g")
            q2g = singles.tile([128, 256], F32, tag="q2g")
            mean1 = singles.tile([128, 256], F32, tag="mean1")
            rstd1 = singles.tile([128, 256], F32, tag="rstd1")
            mb1 = singles.tile([128, 256], F32, tag="mb1")
            mean2 = singles.tile([128, 256], F32, tag="mean2")
            r2mb = singles.tile([128, 256], BF16, tag="r2mb")
            nmrm = singles.tile([128, 256], BF16, tag="nmrm")
            rstd2m = singles.tile([128, 256], F32, tag="rstd2m")
            tmpa = singles.tile([128, 256], F32, tag="tmpa")
            tmpb = singles.tile([128, 256], F32, tag="tmpb")
            tmpc = singles.tile([128, 256], F32, tag="tmpc")
            tmpd = singles.tile([128, 256], F32, tag="tmpd")
            junk = singles.tile([128, 512], BF16, tag="junk")
            junk2 = singles.tile([128, 512], BF16, tag="junk2")
            pooled = singles.tile([H, NU], BF16, tag="pooled")

            mm = nc.tensor.matmul

            # ---- engine routing (tunable via KROUTE env) ----
            ENG = {"dve": nc.vector, "pool": nc.gpsimd}

            def rt_copy(kind, out, in_):
                r = routes[kind]
                if r == "split":
                    n = out.shape[-1]
                    h = ((n + 255) // 256) * 128
                    nc.vector.tensor_copy(out[:, :h], in_[:, :h])
                    if n > h:
                        nc.scalar.activation(out=out[:, h:], in_=in_[:, h:],
                                             func=AF.Copy, bias=0.0, scale=1.0)
                elif r == "act":
                    nc.scalar.activation(out=out, in_=in_, func=AF.Copy,
                                         bias=0.0, scale=1.0)
                else:
                    ENG[r].tensor_copy(out, in_)

            def rt_stt(kind, **kw):
                r = routes[kind]
                if r == "act" and kw.get("op1") == ALU.mult:
                    nc.scalar.activation(out=kw["out"], in_=kw["in0"],
                                         func=AF.Square, bias=0.0, scale=1.0,
                                         accum_out=kw["accum_out"])
                else:
                    ENG[r].scalar_tensor_tensor(**kw)

            def rt_norm(kind, out, in_, mean_c, rstd_c, mbias_c):
                r = routes[kind]
                if r == "act":
                    nc.scalar.activation(out=out, in_=in_, func=AF.Identity,
                                         bias=mbias_c, scale=rstd_c)
                else:
                    ENG[r].tensor_scalar(out=out, in0=in_, scalar1=mean_c,
                                         scalar2=rstd_c, op0=ALU.subtract,
                                         op1=ALU.mult)

            def rt_relu(kind, out, in_):
                r = routes[kind]
                if r == "act":
                    nc.scalar.activation(out=out, in_=in_, func=AF.Relu,
                                         bias=0.0, scale=1.0)
                else:
                    ENG[r].tensor_scalar_max(out=out, in0=in_, scalar1=0.0)

            # chunk geometry helpers
            def cgeom(ch):
                """units with local tile offsets; returns (units, w) where
                units = [(u, base_tile_global, base_local)] and w = #tiles."""
                units = []
                loc = 0
                for u in ch:
                    units.append((u, int(tstart[u]), loc))
                    loc += kt[u]
                return units, loc

            def boundary(u, j):
                """tile j of unit u is its last tile (may be ragged)."""
                return j == kt[u] - 1

            def negb_row(t):
                p = 32 * (t % 3)
                c = (t // 3) * 128
                return negb_sb[p:p + 1, c:c + 128]

            # ================= phase 1 chunk body (exp table) =================
            def emit_p1(ch):
                units, w = cgeom(ch)
                W = w * 128
                t0 = int(tstart[ch[0]])

                xs = xpool.tile([DSEQ, 512], BF16, tag="xs")
                nc.sync.dma_start(out=xs[:, :W], in_=x_in[:, t0 * 128:(t0 + w) * 128])

                emb_ps = pA.tile([128, 512], F32, tag="pA")
                mm(emb_ps[:, :W], w_in, xs[:, :W], start=True, stop=True)
                embT = cp.tile([128, 512], BF16, tag="embT")
                rt_copy("embT", embT[:, :W], emb_ps[:, :W])

                if routes["en"] == "dma":
                    en_sb = cp.tile([128, 512], BF16, tag="en")
                    nc.sync.dma_start_transpose(
                        en_sb[:, :W].rearrange("t (b h) -> t b h", b=w),
                        embT[:, :W])
                else:
                    en_ps = pB.tile([128, 512], F32, tag="pB")
                    for j in range(w):
                        mm(en_ps[:, j * 128:(j + 1) * 128],
                           xs[:, j * 128:(j + 1) * 128], w_in,
                           start=True, stop=True)
                    en_sb = cp.tile([128, 512], BF16, tag="en")
                    rt_copy("enc", en_sb[:, :W], en_ps[:, :W])

                y_ps = pA.tile([128, 512], F32, tag="pA")
                mm(y_ps[:, :W], w_g, embT[:, :W], start=True, stop=True)
                yT = cp.tile([128, 512], BF16, tag="yT")
                rt_copy("yT", yT[:, :W], y_ps[:, :W])

                v_ps = pA.tile([128, 512], F32, tag="pA")
                for j in range(w):
                    mm(v_ps[:, j * 128:(j + 1) * 128],
                       embT[:, j * 128:(j + 1) * 128], w_vo, start=True, stop=True)
                vs = cp.tile([128, 512], BF16, tag="vs")
                rt_copy("vs", vs[:, :W], v_ps[:, :W])

                # scores + exp, one bank per key-tile index (mt)
                maxkt = max(kt[u] for u in ch)
                es_mt = []
                for mt in range(maxkt):
                    # column layout within this mt-bank: q-ranges of units with kt>mt
                    sc_ps = pA.tile([128, 512], F32, tag="pA")
                    col = 0
                    spans = []  # (u, base_local, qw, col)
                    for (u, bg, bl) in units:
                        if kt[u] <= mt:
                            continue
                        qw = kt[u] * 128
                        lhs = embT[:, (bl + mt) * 128:(bl + mt + 1) * 128]
                        bnd = boundary(u, mt)
                        mm(sc_ps[:, col:col + qw], lhs,
                           yT[:, bl * 128:bl * 128 + qw],
                           start=True,
                           stop=(bnd and bool(os.environ.get("KNONEGB"))) or not bnd)
                        if bnd and not os.environ.get("KNONEGB"):
                            p3 = 32 * ((bg + mt) % 3)
                            mm(sc_ps[:, col:col + qw], negb_row(bg + mt),
                               ones_row[p3:p3 + 1, :qw], start=False, stop=True)
                        spans.append((u, bl, qw, col))
                        col += qw
                    es = esp.tile([128, 512], BF16, tag=f"es{mt}")
                    nc.scalar.activation(out=es[:, :col], in_=sc_ps[:, :col],
                                         func=AF.Exp, bias=0.0, scale=CSCALE)
                    es_mt.append((es, spans))

                # fused: den (exp row-sums) and ao_proj = es.T @ (emb @ Wv Wo)
                # share the same stationary es tile per (unit, qtile, ktile)
                den_ps = pS.tile([128, 16], F32, tag="sm")
                pon_ps = pB.tile([128, 512], F32, tag="pB")
                for (u, bg, bl) in units:
                    for j in range(kt[u]):
                        for mt in range(kt[u]):
                            es, spans = es_mt[mt]
                            ucol = next(c for (uu, _, _, c) in spans if uu == u)
                            lhs = es[:, ucol + j * 128:ucol + (j + 1) * 128]
                            st, sp = (mt == 0), (mt == kt[u] - 1)
                            mm(den_ps[:, bl + j:bl + j + 1], lhs, ones_col,
                               start=st, stop=sp)
                            mm(pon_ps[:, (bl + j) * 128:(bl + j + 1) * 128],
                               lhs, vs[:, (bl + mt) * 128:(bl + mt + 1) * 128],
                               start=st, stop=sp)

                if os.environ.get("KDENSB", "0") == "1":
                    den_sb = cp.tile([128, 16], F32, tag="den_sb")
                    nc.vector.tensor_copy(den_sb[:, :w], den_ps[:, :w])
                else:
                    den_sb = den_ps
                # x1in = den*emb + ao_proj  (token-major), stats via accum
                for j in range(w):
                    t = t0 + j
                    xsl = x1in_all[:, t * 128:(t + 1) * 128]
                    rt_stt("stt1", out=xsl, in0=en_sb[:, j * 128:(j + 1) * 128],
                           scalar=den_sb[:, j:j + 1],
                           in1=pon_ps[:, j * 128:(j + 1) * 128],
                           op0=ALU.mult, op1=ALU.add,
                           accum_out=s1g[:, t:t + 1])
                    rt_stt("sq1", out=junk[:, (t % 4) * 128:(t % 4 + 1) * 128],
                           in0=xsl, scalar=1.0, in1=xsl,
                           op0=ALU.mult, op1=ALU.mult,
                           accum_out=q1g[:, t:t + 1])

            # ---- batched LN1 stats: rstd1, mb1 = -mean*rstd ----
            def emit_stats1(ta, tb):
                nc.vector.tensor_scalar(out=mean1[:, ta:tb], in0=s1g[:, ta:tb],
                                        scalar1=1.0 / H, scalar2=None, op0=ALU.mult)
                nc.vector.tensor_tensor(out=tmpa[:, ta:tb], in0=mean1[:, ta:tb],
                                        in1=mean1[:, ta:tb], op=ALU.mult)
                nc.vector.scalar_tensor_tensor(
                    out=tmpb[:, ta:tb], in0=q1g[:, ta:tb], scalar=1.0 / H,
                    in1=tmpa[:, ta:tb], op0=ALU.mult, op1=ALU.subtract)
                nc.scalar.activation(out=tmpa[:, ta:tb], in_=tmpb[:, ta:tb],
                                     func=AF.Sqrt, bias=eps_col, scale=1.0)
                nc.vector.reciprocal(rstd1[:, ta:tb], tmpa[:, ta:tb])
                nc.vector.scalar_tensor_tensor(
                    out=mb1[:, ta:tb], in0=mean1[:, ta:tb], scalar=-1.0,
                    in1=rstd1[:, ta:tb], op0=ALU.mult, op1=ALU.mult)

            # ================= phase 2 supergroup (sqrt table) =================
            def emit_p2(sg0):
                sgch = chunks[sg0:sg0 + SGC]
                # B2: normalize, FFN, residual + stats
                for si, ch in enumerate(sgch):
                    units, w = cgeom(ch)
                    W = w * 128
                    t0 = int(tstart[ch[0]])
                    x1 = wk2.tile([128, 512], BF16, tag="x1")
                    for j in range(w):
                        t = t0 + j
                        rt_norm("norm1", x1[:, j * 128:(j + 1) * 128],
                                x1in_all[:, t * 128:(t + 1) * 128],
                                mean1[:, t:t + 1], rstd1[:, t:t + 1],
                                mb1[:, t:t + 1])
                    x1T = wk2.tile([128, 512], BF16, tag="x1T")
                    if routes.get("x1t", "dma") == "dma":
                        nc.sync.dma_start_transpose(
                            x1T[:, :W].rearrange("h (b t) -> h b t", b=w),
                            x1[:, :W])
                    else:
                        x1t_ps = pTT.tile([128, 512], BF16, tag="pBt")
                        for j in range(w):
                            nc.tensor.transpose(
                                x1t_ps[:, j * 128:(j + 1) * 128],
                                x1[:, j * 128:(j + 1) * 128], identb)
                        rt_copy("x1tc", x1T[:, :W], x1t_ps[:, :W])

                    f1_ps = pA.tile([128, 512], F32, tag="pA")
                    mm(f1_ps[:, :W], w_f1, x1T[:, :W], start=True, stop=True)
                    f1r = wk2.tile([128, 512], BF16, tag="f1r")
                    rt_relu("relu", f1r[:, :W], f1_ps[:, :W])

                    f2_ps = pB.tile([128, 512], F32, tag="pB")
                    s2_ps = pS.tile([128, 16], F32, tag="sm")
                    for j in range(w):
                        f1sl = f1r[:, j * 128:(j + 1) * 128]
                        x1sl = x1T[:, j * 128:(j + 1) * 128]
                        mm(f2_ps[:, j * 128:(j + 1) * 128], f1sl, w_f2,
                           start=True, stop=False)
                        mm(f2_ps[:, j * 128:(j + 1) * 128], x1sl, identb,
                           start=False, stop=True)
                        mm(s2_ps[:, j:j + 1], f1sl, c2col, start=True, stop=False)
                        mm(s2_ps[:, j:j + 1], x1sl, ones_col, start=False, stop=True)
                    if os.environ.get("KSMC", "act") == "act":
                        nc.scalar.activation(out=s2g[:, t0:t0 + w], in_=s2_ps[:, :w],
                                             func=AF.Copy, bias=0.0, scale=1.0)
                    else:
                        nc.vector.tensor_copy(s2g[:, t0:t0 + w], s2_ps[:, :w])
                    xsl = x2in_sg[:, si * 512:si * 512 + W]
                    rt_copy("x2c", xsl, f2_ps[:, :W])
                    for j in range(w):
                        t = t0 + j
                        xj = x2in_sg[:, (si * 4 + j) * 128:(si * 4 + j + 1) * 128]
                        rt_stt("sq2", out=junk2[:, (t % 4) * 128:(t % 4 + 1) * 128],
                               in0=xj, scalar=1.0,
                               in1=xj, op0=ALU.mult, op1=ALU.mult,
                               accum_out=q2g[:, t:t + 1])

                # batched LN2 stats for this supergroup
                ta = int(tstart[sgch[0][0]])
                tb = int(tstart[sgch[-1][-1]]) + kt[sgch[-1][-1]]
                nt = tb - ta
                nc.vector.tensor_scalar(out=mean2[:, ta:tb], in0=s2g[:, ta:tb],
                                        scalar1=1.0 / H, scalar2=None, op0=ALU.mult)
                nc.vector.tensor_tensor(out=tmpc[:, ta:tb], in0=mean2[:, ta:tb],
                                        in1=mean2[:, ta:tb], op=ALU.mult)
                nc.vector.scalar_tensor_tensor(
                    out=tmpd[:, ta:tb], in0=q2g[:, ta:tb], scalar=1.0 / H,
                    in1=tmpc[:, ta:tb], op0=ALU.mult, op1=ALU.subtract)
                nc.scalar.activation(out=tmpc[:, ta:tb], in_=tmpd[:, ta:tb],
                                     func=AF.Sqrt, bias=eps_col, scale=1.0)
                nc.vector.reciprocal(tmpd[:, ta:tb], tmpc[:, ta:tb])
                nc.vector.tensor_tensor(out=rstd2m[:, ta:tb], in0=tmpd[:, ta:tb],
                                        in1=qm_sb[:, ta:tb], op=ALU.mult)
                nc.vector.tensor_copy(r2mb[:, ta:tb], rstd2m[:, ta:tb])
                nc.vector.scalar_tensor_tensor(
                    out=nmrm[:, ta:tb], in0=mean2[:, ta:tb], scalar=-1.0,
                    in1=rstd2m[:, ta:tb], op0=ALU.mult, op1=ALU.mult)

                # B3: normalize + masked sum-pool
                for si, ch in enumerate(sgch):
                    units, w = cgeom(ch)
                    t0 = int(tstart[ch[0]])
                    pool_ps = pS.tile([128, 16], F32, tag="sm")
                    for ui, (u, bg, bl) in enumerate(units):
                        for j in range(kt[u]):
                            t = t0 + bl + j
                            x2sl = x2in_sg[:, (si * 4 + bl + j) * 128:
                                           (si * 4 + bl + j + 1) * 128]
                            mm(pool_ps[:, ui:ui + 1], x2sl, r2mb[:, t:t + 1],
                               start=(j == 0), stop=False)
                            mm(pool_ps[:, ui:ui + 1], ones_mat,
                               nmrm[:, t:t + 1],
                               start=False, stop=(j == kt[u] - 1))
                    if os.environ.get("KSMC", "act") == "act":
                        nc.scalar.activation(
                            out=pooled[:, ch[0]:ch[0] + len(units)],
                            in_=pool_ps[:, :len(units)],
                            func=AF.Copy, bias=0.0, scale=1.0)
                    else:
                        nc.vector.tensor_copy(
                            pooled[:, ch[0]:ch[0] + len(units)],
                            pool_ps[:, :len(units)])

            # ================= interleaved driver =================
            def sg_trange(sg0):
                sgch = chunks[sg0:sg0 + SGC]
                ta = int(tstart[sgch[0][0]])
                tb = int(tstart[sgch[-1][-1]]) + kt[sgch[-1][-1]]
                return ta, tb

            LAG = int(os.environ.get("KLAG", "0"))
            sgs = list(range(0, NCH, SGC))
            if LAG == 0:
                for ch in chunks:
                    emit_p1(ch)
                for sg0 in sgs:
                    emit_stats1(*sg_trange(sg0))
                    emit_p2(sg0)
            else:
                for i, sg0 in enumerate(sgs):
                    for ch in chunks[sg0:sg0 + SGC]:
                        emit_p1(ch)
                    if i >= LAG:
                        prev = sgs[i - LAG]
                        emit_stats1(*sg_trange(prev))
                        emit_p2(prev)
                for i in range(max(0, len(sgs) - LAG), len(sgs)):
                    emit_stats1(*sg_trange(sgs[i]))
                    emit_p2(sgs[i])

            # ================= tail: unit_fc, building sum, fusion =================
            u16_ps = pB.tile([UNITD, NU], F32, tag="pB")
            mm(u16_ps, w_u, pooled, start=True, stop=True)
            u16 = cp.tile([UNITD, NU], F32, tag="u16")
            nc.scalar.activation(out=u16, in_=u16_ps, func=AF.Relu,
                                 bias=0.0, scale=1.0)

            u16t_ps = pB.tile([NU, UNITD], F32, tag="pB")
            nc.tensor.transpose(u16t_ps, u16, identf[:UNITD, :UNITD])
            u16t = cp.tile([NU, UNITD], BF16, tag="u16t")
            nc.vector.tensor_copy(u16t, u16t_ps)

            seq_ps = pB.tile([UNITD, BPC], F32, tag="pB")
            mm(seq_ps, u16t, s_sb, start=True, stop=True)

            fused = cp.tile([UNITD + AGGD + TODD, BPC], BF16, tag="fused")
            nc.vector.tensor_copy(fused[:UNITD, :], seq_ps)
            nc.sync.dma_start(out=fused[UNITD:, :], in_=tail_in[:, :])

            h1_ps = pB.tile([H, BPC], F32, tag="pB")
            mm(h1_ps, w_c1, fused, start=True, stop=True)
            h1 = cp.tile([H, BPC], BF16, tag="h1")
            nc.scalar.activation(out=h1, in_=h1_ps, func=AF.Relu,
                                 bias=0.0, scale=1.0)

            o_ps = pB.tile([DOUT, BPC], F32, tag="pB")
            mm(o_ps, w_c2, h1, start=True, stop=True)
            o_s = cp.tile([DOUT, BPC], F32, tag="osb")
            nc.scalar.activation(out=o_s, in_=o_ps, func=AF.Relu,
                                 bias=0.0, scale=1.0)
            nc.sync.dma_start(out=out_t[:, :], in_=o_s)

    return nc


def _prep_weights(inputs):
    ipw = np.asarray(inputs["in_proj_w"])
    wts = {
        "w_inT": np.asarray(inputs["W_in"]).T,        # [5,128]
        "w_gT": (ipw[0:H] @ ipw[H:2 * H].T),          # composed q/k [128,128]
        "w_voT": ipw[2 * H:3 * H].T @ np.asarray(inputs["out_proj_w"]).T,
        "w_f1T": np.asarray(inputs["W_ff1"]).T,
        "w_f2T": np.asarray(inputs["W_ff2"]).T,
        "w_c2col": np.asarray(inputs["W_ff2"]).T.sum(axis=1, keepdims=True),
        "w_uT": np.asarray(inputs["W_unit"]).T,       # [128,16]
        "w_c1T": np.asarray(inputs["W_fc1"]).T,       # [26,128]
        "w_c2T": np.asarray(inputs["W_fc2"]).T,       # [128,128]
    }
    wts = {k: np.ascontiguousarray(v.astype(NPBF)) for k, v in wts.items()}
    for nm in ("b_in", "in_proj_b", "out_proj_b", "b_ff1", "b_ff2",
               "ln1_b", "ln2_b", "b_unit", "b_fc1", "b_fc2"):
        assert np.max(np.abs(np.asarray(inputs[nm]))) == 0.0, f"{nm} nonzero"
    for nm in ("ln1_w", "ln2_w"):
        assert np.allclose(np.asarray(inputs[nm]), 1.0), f"{nm} nontrivial"

    lengths = np.asarray(inputs["lengths"]).reshape(NCORES, NU)
    nlong = int(max((lengths[c] > 128).sum() for c in range(NCORES)))
    wts["_nlong"] = nlong
    return wts


def make_in_maps(inputs, nlong):
    x_seq = np.asarray(inputs["x_seq"], dtype=np.float32)        # [B,U,L,5]
    lengths = np.asarray(inputs["lengths"]).reshape(NCORES, NU)
    x_agg = np.asarray(inputs["x_agg_quant"], dtype=np.float32)  # [B,7]
    tod_emb = np.asarray(inputs["tod_emb"], dtype=np.float32)    # [5,3]
    tod_idx = np.asarray(inputs["tod_idx"])                      # [B]

    kt_mod = np.array([1] * (NU - nlong) + [2] * nlong)
    tstart = np.concatenate([[0], np.cumsum(kt_mod)]).astype(int)
    TT = int(tstart[NU])

    in_maps = []
    for c in range(NCORES):
        lens = lengths[c]
        xc = x_seq[c * BPC:(c + 1) * BPC].reshape(NU, L, DSEQ)
        # sort units: shorts (len<=128) first
        order = np.argsort(lens > 128, kind="stable")
        xg = np.zeros((DSEQ, TT * 128), np.float32)
        NB3 = (TT + 2) // 3
        negbT = np.zeros((65, NB3 * 128), np.float32)
        qm01 = np.zeros((128, 256), np.float32)
        for i in range(NU):
            u = order[i]
            ln = int(lens[u])
            t0 = int(tstart[i])
            ntile = int(kt_mod[i])
            for j in range(ntile):
                t = t0 + j
                lo = j * 128
                valid = max(0, min(128, ln - lo))
                if valid > 0:
                    xg[:, t * 128:t * 128 + valid] = \
                        xc[u, lo:lo + valid, :].T
                negbT[32 * (t % 3), (t // 3) * 128 + valid:(t // 3 + 1) * 128] = NEGB
                qm01[:valid, t] = 1.0
        S = np.zeros((NU, BPC), np.float32)
        S[np.arange(NU), order // U] = 1.0
        tail = np.concatenate(
            [x_agg[c * BPC:(c + 1) * BPC].T,
             tod_emb[tod_idx[c * BPC:(c + 1) * BPC]].T], axis=0)
        in_maps.append({
            "xg": np.ascontiguousarray(xg).astype(NPBF),
            "negbT": np.ascontiguousarray(negbT).astype(NPBF),
            "qm01": np.ascontiguousarray(qm01),
            "S": S.astype(NPBF),
            "tail": np.ascontiguousarray(tail).astype(NPBF),
        })
    return in_maps


def kernel(_trace=False, **inputs):
    wts = _prep_weights(inputs)
    nlong = wts["_nlong"]
    nc = build_nc(wts)
    if not nc.is_finalized():
        nc.finalize()
    in_maps = make_in_maps(inputs, nlong)
    res = run_bass_kernel_spmd(nc, in_maps, core_ids=list(range(NCORES)),
                               trace=_trace)
    out = np.zeros((B, DOUT), np.float32)
    for c in range(NCORES):
        out[c * BPC:(c + 1) * BPC, :] = res.results[c]["outT"].T
    if _trace:
        kernel._last_results = res
    return out
